# revision 6
# baseline (speedup 1.0000x reference)
"""Trainium2 Bass kernel for the pairwise-MLP + power-iteration module.

Computation (see host reference):
  - For each of P=256 (s,t) node pairs and B=8 graphs, build a 256-d feature row
    (a flat reinterpretation of stacked s/t embeddings), run a 256->4096->4096
    ->4096->256 LeakyReLU MLP with a final sigmoid -> a 16x16 positive matrix.
  - Power-iterate each matrix to its Perron eigenvector, divide by the source
    component, scale by Ts[pair], and sum contributions over all pairs -> (B, N).

Distribution: data-parallel over the P axis; each of the 8 cores runs 32 pairs
x 8 graphs = 256 MLP rows. Activations stay SBUF-resident in transposed layout
(hidden on partitions, rows on the free axis); W2/W3 stream from HBM in
column-stripes. Matmuls run in float32r (TF32-like multiply, fp32 accumulate).
The per-core [256 rows, 16] contribution block is returned and the final
pair/core reduction happens on the host (the all-reduce step).
"""

import numpy as np
from contextlib import ExitStack

import concourse.bass as bass
import concourse.tile as tile
from concourse import bacc, mybir
from concourse.bass_utils import run_bass_kernel_spmd

B = 8
NN = 16
D = 128
H = 4096
P = NN * NN            # 256 pairs
NCORES = 8
RPC = P * B // NCORES  # 256 rows per core
SLOPE = 0.01
PI_ITERS = 12

F32 = mybir.dt.float32

_cache = {}


def _build_program(dt_mm, pi_iters=PI_ITERS, sim_safe=False):
    KT1 = 2 * D // 128   # 2 k-tiles for layer 1
    KT = H // 128        # 32 k-tiles for layers 2-4
    MT = H // 128        # 32 m-stripes for layers 1-3

    nc = bacc.Bacc("TRN2", target_bir_lowering=False, debug=False,
                   num_devices=NCORES)

    def emit_lrelu(pool, out_ap, in_ap, bias):
        """out = LeakyReLU(in + bias). CoreSim lacks Lrelu, so the sim build
        decomposes it as pre*alpha + relu(pre)*(1-alpha)."""
        if not sim_safe:
            nc.scalar.activation(out_ap, in_ap, mybir.ActivationFunctionType.Lrelu,
                                 bias=bias, scale=1.0, alpha=SLOPE)
            return
        shape = [in_ap.shape[0], in_ap.free_size()]
        pre = pool.tile(shape, F32, tag="lr_pre")
        nc.scalar.activation(pre[:], in_ap, mybir.ActivationFunctionType.Identity,
                             bias=bias, scale=1.0)
        pos = pool.tile(shape, F32, tag="lr_pos")
        nc.scalar.activation(pos[:], pre[:], mybir.ActivationFunctionType.Relu)
        t1 = pool.tile(shape, F32, tag="lr_t1")
        nc.vector.tensor_scalar_mul(t1[:], pos[:], 1.0 - SLOPE)
        nc.vector.scalar_tensor_tensor(out_ap, pre[:], SLOPE, t1[:],
                                       op0=mybir.AluOpType.mult,
                                       op1=mybir.AluOpType.add)

    xt_d = nc.dram_tensor("xt", [2 * D, RPC], dt_mm, kind="ExternalInput").ap()
    w1_d = nc.dram_tensor("w1", [2 * D, H], dt_mm, kind="ExternalInput").ap()
    w2_d = nc.dram_tensor("w2", [H, H], dt_mm, kind="ExternalInput").ap()
    w3_d = nc.dram_tensor("w3", [H, H], dt_mm, kind="ExternalInput").ap()
    w4_d = nc.dram_tensor("w4", [H, P], dt_mm, kind="ExternalInput").ap()
    b1_d = nc.dram_tensor("b1", [H], F32, kind="ExternalInput").ap()
    b2_d = nc.dram_tensor("b2", [H], F32, kind="ExternalInput").ap()
    b3_d = nc.dram_tensor("b3", [H], F32, kind="ExternalInput").ap()
    b4r_d = nc.dram_tensor("b4r", [128, P], F32, kind="ExternalInput").ap()
    trow_d = nc.dram_tensor("trow", [RPC], F32, kind="ExternalInput").ap()
    msk_d = nc.dram_tensor("msk", [RPC, NN], F32, kind="ExternalInput").ap()
    out_d = nc.dram_tensor("contrib", [RPC, NN], F32, kind="ExternalOutput").ap()

    with tile.TileContext(nc) as tc, ExitStack() as ctx:
        const = ctx.enter_context(tc.tile_pool(name="const", bufs=1))
        hpool = ctx.enter_context(tc.tile_pool(name="h", bufs=2))
        wstream = ctx.enter_context(tc.tile_pool(name="wstream", bufs=3))
        small = ctx.enter_context(tc.tile_pool(name="small", bufs=2))
        pipool = ctx.enter_context(tc.tile_pool(name="pi", bufs=2))
        ps = ctx.enter_context(tc.tile_pool(name="ps", bufs=4, space="PSUM"))

        # ---- resident loads ----
        xt_t = const.tile([128, KT1, RPC], dt_mm, tag="xt")
        nc.sync.dma_start(out=xt_t[:], in_=xt_d.rearrange("(k p) r -> p k r", p=128))
        w1_t = const.tile([128, KT1, H], dt_mm, tag="w1")
        nc.sync.dma_start(out=w1_t[:], in_=w1_d.rearrange("(k p) h -> p k h", p=128))
        w4_t = const.tile([128, KT, P], dt_mm, tag="w4")
        nc.sync.dma_start(out=w4_t[:], in_=w4_d.rearrange("(k p) u -> p k u", p=128))
        b_t = {}
        for nm, bd in [("b1", b1_d), ("b2", b2_d), ("b3", b3_d)]:
            t = const.tile([128, MT], F32, tag=nm)
            nc.sync.dma_start(out=t[:], in_=bd.rearrange("(m p) -> p m", p=128))
            b_t[nm] = t
        b4r_t = const.tile([128, P], F32, tag="b4r")
        nc.sync.dma_start(out=b4r_t[:], in_=b4r_d[:])
        msk_t = const.tile([128, 2, NN], F32, tag="msk")
        nc.sync.dma_start(out=msk_t[:], in_=msk_d.rearrange("(g p) n -> p g n", p=128))
        trow_t = const.tile([128, 2], F32, tag="trow")
        nc.sync.dma_start(out=trow_t[:], in_=trow_d.rearrange("(g p) -> p g", p=128))

        # ---- MLP layers 1-3 (output transposed: hidden on partitions) ----
        w2_r = w2_d.rearrange("(k p) (m c) -> p k m c", p=128, c=128)
        w3_r = w3_d.rearrange("(k p) (m c) -> p k m c", p=128, c=128)

        h1_t = hpool.tile([128, MT, RPC], dt_mm, tag="h")
        for m in range(MT):
            acc = ps.tile([128, RPC], F32, tag="acc")
            for k in range(KT1):
                nc.tensor.matmul(acc[:], w1_t[:, k, m * 128:(m + 1) * 128],
                                 xt_t[:, k, :], start=(k == 0), stop=(k == KT1 - 1))
            emit_lrelu(small, h1_t[:, m, :], acc[:], b_t["b1"][:, m:m + 1])

        h2_t = hpool.tile([128, MT, RPC], dt_mm, tag="h")
        for m in range(MT):
            ws = wstream.tile([128, KT, 128], dt_mm, tag="ws")
            nc.sync.dma_start(out=ws[:], in_=w2_r[:, :, m, :])
            acc = ps.tile([128, RPC], F32, tag="acc")
            for k in range(KT):
                nc.tensor.matmul(acc[:], ws[:, k, :], h1_t[:, k, :],
                                 start=(k == 0), stop=(k == KT - 1))
            emit_lrelu(small, h2_t[:, m, :], acc[:], b_t["b2"][:, m:m + 1])

        h3_t = hpool.tile([128, MT, RPC], dt_mm, tag="h")
        for m in range(MT):
            ws = wstream.tile([128, KT, 128], dt_mm, tag="ws")
            nc.sync.dma_start(out=ws[:], in_=w3_r[:, :, m, :])
            acc = ps.tile([128, RPC], F32, tag="acc")
            for k in range(KT):
                nc.tensor.matmul(acc[:], ws[:, k, :], h2_t[:, k, :],
                                 start=(k == 0), stop=(k == KT - 1))
            emit_lrelu(small, h3_t[:, m, :], acc[:], b_t["b3"][:, m:m + 1])

        # ---- layer 4 (swapped operands: rows on partitions) + power iteration ----
        for g in range(RPC // 128):
            accA = ps.tile([128, P], F32, tag="acc")
            for k in range(KT):
                nc.tensor.matmul(accA[:], h3_t[:, k, g * 128:(g + 1) * 128],
                                 w4_t[:, k, :], start=(k == 0), stop=(k == KT - 1))
            pre = small.tile([128, P], F32, tag="pre")
            nc.vector.tensor_add(pre[:], accA[:], b4r_t[:])
            lr = small.tile([128, P], F32, tag="lr")
            emit_lrelu(small, lr[:], pre[:], 0.0)
            A_t = small.tile([128, P], F32, tag="A")
            nc.scalar.activation(A_t[:], lr[:], mybir.ActivationFunctionType.Sigmoid)
            A3 = A_t[:].rearrange("p (i j) -> p i j", j=NN)

            # power iteration: v0 = ones; per-step max-normalization is folded
            # into the next multiply (the final vec/v_src ratio is
            # scale-invariant, so any per-step scaling is valid).
            v_t = pipool.tile([128, NN], F32, tag="v")
            nc.vector.memset(v_t[:], 1.0)
            inv_t = pipool.tile([128, 1], F32, tag="inv")
            nc.vector.memset(inv_t[:], 1.0)
            for it in range(pi_iters):
                v_b = v_t[:].rearrange("p (a j) -> p a j", a=1).to_broadcast((128, NN, NN))
                prod = pipool.tile([128, NN, NN], F32, tag="prod")
                nc.vector.scalar_tensor_tensor(
                    prod[:], A3, inv_t[:, 0:1], v_b,
                    op0=mybir.AluOpType.mult, op1=mybir.AluOpType.mult)
                w_t = pipool.tile([128, NN], F32, tag="v")
                nc.vector.tensor_reduce(w_t[:], prod[:], axis=mybir.AxisListType.X,
                                        op=mybir.AluOpType.add)
                if it < pi_iters - 1:
                    mx = pipool.tile([128, 1], F32, tag="mx")
                    nc.vector.tensor_reduce(mx[:], w_t[:], axis=mybir.AxisListType.X,
                                            op=mybir.AluOpType.max)
                    inv_t = pipool.tile([128, 1], F32, tag="inv")
                    nc.vector.reciprocal(inv_t[:], mx[:])
                v_t = w_t

            # contrib = vec / vec[src] * T
            dummy = pipool.tile([128, NN], F32, tag="dummy")
            nc.vector.tensor_mul(dummy[:], v_t[:], msk_t[:, g, :])
            vsrc = pipool.tile([128, 1], F32, tag="vsrc")
            nc.vector.tensor_reduce(vsrc[:], dummy[:], axis=mybir.AxisListType.X,
                                    op=mybir.AluOpType.add)
            rsrc = pipool.tile([128, 1], F32, tag="rsrc")
            nc.vector.reciprocal(rsrc[:], vsrc[:])
            rt = pipool.tile([128, 1], F32, tag="rt")
            nc.vector.tensor_mul(rt[:], rsrc[:], trow_t[:, g:g + 1])
            contrib = pipool.tile([128, NN], F32, tag="contrib")
            nc.vector.tensor_scalar_mul(contrib[:], v_t[:], rt[:, 0:1])
            nc.sync.dma_start(out=out_d[g * 128:(g + 1) * 128, :], in_=contrib[:])

    nc.compile()
    return nc


def _np_dtype(dt_mm):
    import ml_dtypes
    if dt_mm == mybir.dt.bfloat16:
        return np.dtype(ml_dtypes.bfloat16)
    return np.float32


def _host_prep(inputs, dt_mm):
    """Build per-core input maps. Pure layout/indexing work."""
    ndt = _np_dtype(dt_mm)
    emb = np.asarray(inputs["nodes_embeddings"], dtype=np.float32)   # (B, N, D)
    Ts = np.asarray(inputs["Ts"], dtype=np.float32)                  # (B, N, N)
    W1 = np.ascontiguousarray(np.asarray(inputs["W1"], np.float32)).astype(ndt)
    W2 = np.ascontiguousarray(np.asarray(inputs["W2"], np.float32)).astype(ndt)
    W3 = np.ascontiguousarray(np.asarray(inputs["W3"], np.float32)).astype(ndt)
    W4 = np.ascontiguousarray(np.asarray(inputs["W4"], np.float32)).astype(ndt)
    b1 = np.ascontiguousarray(np.asarray(inputs["b1"], np.float32))
    b2 = np.ascontiguousarray(np.asarray(inputs["b2"], np.float32))
    b3 = np.ascontiguousarray(np.asarray(inputs["b3"], np.float32))
    b4 = np.asarray(inputs["b4"], np.float32)
    b4r = np.ascontiguousarray(np.broadcast_to(b4[None, :], (128, P)))

    embT = emb.transpose(1, 0, 2)                       # (N, B, D)
    pair = np.arange(P)
    s_ids = pair // NN
    t_ids = pair % NN
    # faithful to the reference: stack then flat-reinterpret, NOT per-row concat
    x = np.stack([embT[s_ids], embT[t_ids]], axis=1).reshape(P, B, 2 * D)
    xr = x.reshape(P * B, 2 * D)                        # (2048, 256)

    T_pair = Ts.reshape(B, P).T                         # (P, B)
    msk_full = (np.arange(NN)[None, :] == s_ids[:, None]).astype(np.float32)  # (P, NN)

    in_maps = []
    ppc = P // NCORES                                   # 32 pairs per core
    for c in range(NCORES):
        rows = slice(c * RPC, (c + 1) * RPC)
        prs = slice(c * ppc, (c + 1) * ppc)
        xt_c = np.ascontiguousarray(xr[rows].T).astype(ndt)          # (256, RPC)
        trow_c = np.ascontiguousarray(T_pair[prs, :].reshape(RPC))   # (RPC,)
        msk_c = np.ascontiguousarray(np.repeat(msk_full[prs], B, axis=0))  # (RPC, NN)
        in_maps.append({
            "xt": xt_c, "w1": W1, "w2": W2, "w3": W3, "w4": W4,
            "b1": b1, "b2": b2, "b3": b3, "b4r": b4r,
            "trow": trow_c, "msk": msk_c,
        })
    return in_maps


def _get_program(dt_mm, sim_safe=False):
    key = (str(dt_mm), sim_safe)
    if key not in _cache:
        _cache[key] = _build_program(dt_mm, sim_safe=sim_safe)
    return _cache[key]


def run(inputs, dt_mm=mybir.dt.float32r, trace=False):
    nc = _get_program(dt_mm)
    in_maps = _host_prep(inputs, dt_mm)
    res = run_bass_kernel_spmd(nc, in_maps, list(range(NCORES)), trace=trace)
    contribs = [res.results[c]["contrib"].astype(np.float32) for c in range(NCORES)]
    # unshard: sum pair contributions within each core block, then across cores
    out = np.zeros((B, NN), dtype=np.float64)
    for c in range(NCORES):
        out += contribs[c].astype(np.float64).reshape(P // NCORES, B, NN).sum(axis=0)
    return out.astype(np.float32), res


def kernel(**inputs):
    out, _ = run(inputs)
    return out


# revision 7
# speedup vs baseline: 1.4167x; 1.4167x over previous
"""Trainium2 Bass kernel for the pairwise-MLP + power-iteration module.

Computation (see host reference):
  - For each of P=256 (s,t) node pairs and B=8 graphs, build a 256-d feature row
    (a flat reinterpretation of stacked s/t embeddings), run a 256->4096->4096
    ->4096->256 LeakyReLU MLP with a final sigmoid -> a 16x16 positive matrix.
  - Power-iterate each matrix to its Perron eigenvector, divide by the source
    component, scale by Ts[pair], and sum contributions over all pairs -> (B, N).

Distribution: data-parallel over the P axis; each of the 8 cores runs 32 pairs
x 8 graphs = 256 MLP rows. Activations stay SBUF-resident in transposed layout
(hidden on partitions, rows on the free axis); W2/W3 stream from HBM in
column-stripes. Matmuls run in float32r (TF32-like multiply, fp32 accumulate).
The per-core [256 rows, 16] contribution block is returned and the final
pair/core reduction happens on the host (the all-reduce step).
"""

import numpy as np
from contextlib import ExitStack

import concourse.bass as bass
import concourse.tile as tile
from concourse import bacc, mybir
from concourse.bass_utils import run_bass_kernel_spmd

B = 8
NN = 16
D = 128
H = 4096
P = NN * NN            # 256 pairs
NCORES = 8
RPC = P * B // NCORES  # 256 rows per core
SLOPE = 0.01
PI_ITERS = 12

F32 = mybir.dt.float32

_cache = {}


def _build_program(dt_mm, pi_iters=PI_ITERS, sim_safe=False, l2_reps=1):
    KT1 = 2 * D // 128   # 2 k-tiles for layer 1
    KT = H // 128        # 32 k-tiles for layers 2-4
    MT = H // 128        # 32 m-stripes for layers 1-3

    nc = bacc.Bacc("TRN2", target_bir_lowering=False, debug=False,
                   num_devices=NCORES)

    def emit_lrelu(pool, out_ap, in_ap, bias):
        """out = LeakyReLU(in + bias). CoreSim lacks Lrelu, so the sim build
        decomposes it as pre*alpha + relu(pre)*(1-alpha)."""
        if not sim_safe:
            nc.scalar.activation(out_ap, in_ap, mybir.ActivationFunctionType.Lrelu,
                                 bias=bias, scale=1.0, alpha=SLOPE)
            return
        shape = [in_ap.shape[0], in_ap.free_size()]
        pre = pool.tile(shape, F32, tag="lr_pre")
        nc.scalar.activation(pre[:], in_ap, mybir.ActivationFunctionType.Identity,
                             bias=bias, scale=1.0)
        pos = pool.tile(shape, F32, tag="lr_pos")
        nc.scalar.activation(pos[:], pre[:], mybir.ActivationFunctionType.Relu)
        t1 = pool.tile(shape, F32, tag="lr_t1")
        nc.vector.tensor_scalar_mul(t1[:], pos[:], 1.0 - SLOPE)
        nc.vector.scalar_tensor_tensor(out_ap, pre[:], SLOPE, t1[:],
                                       op0=mybir.AluOpType.mult,
                                       op1=mybir.AluOpType.add)

    xt_d = nc.dram_tensor("xt", [2 * D, RPC], dt_mm, kind="ExternalInput").ap()
    w1_d = nc.dram_tensor("w1", [2 * D, H], dt_mm, kind="ExternalInput").ap()
    w2_d = nc.dram_tensor("w2", [H, H], dt_mm, kind="ExternalInput").ap()
    w3_d = nc.dram_tensor("w3", [H, H], dt_mm, kind="ExternalInput").ap()
    w4_d = nc.dram_tensor("w4", [H, P], dt_mm, kind="ExternalInput").ap()
    b1_d = nc.dram_tensor("b1", [H], F32, kind="ExternalInput").ap()
    b2_d = nc.dram_tensor("b2", [H], F32, kind="ExternalInput").ap()
    b3_d = nc.dram_tensor("b3", [H], F32, kind="ExternalInput").ap()
    b4r_d = nc.dram_tensor("b4r", [128, P], F32, kind="ExternalInput").ap()
    trow_d = nc.dram_tensor("trow", [RPC], F32, kind="ExternalInput").ap()
    msk_d = nc.dram_tensor("msk", [RPC, NN], F32, kind="ExternalInput").ap()
    out_d = nc.dram_tensor("contrib", [RPC, NN], F32, kind="ExternalOutput").ap()

    with tile.TileContext(nc) as tc, ExitStack() as ctx:
        const = ctx.enter_context(tc.tile_pool(name="const", bufs=1))
        hpool = ctx.enter_context(tc.tile_pool(name="h", bufs=2))
        wstream = ctx.enter_context(tc.tile_pool(name="wstream", bufs=3))
        small = ctx.enter_context(tc.tile_pool(name="small", bufs=2))
        pipool = ctx.enter_context(tc.tile_pool(name="pi", bufs=2))
        ps = ctx.enter_context(tc.tile_pool(name="ps", bufs=4, space="PSUM"))

        # ---- resident loads ----
        xt_t = const.tile([128, KT1, RPC], dt_mm, tag="xt")
        nc.sync.dma_start(out=xt_t[:], in_=xt_d.rearrange("(k p) r -> p k r", p=128))
        w1_t = const.tile([128, KT1, H], dt_mm, tag="w1")
        nc.sync.dma_start(out=w1_t[:], in_=w1_d.rearrange("(k p) h -> p k h", p=128))
        w4_t = const.tile([128, KT, P], dt_mm, tag="w4")
        nc.sync.dma_start(out=w4_t[:], in_=w4_d.rearrange("(k p) u -> p k u", p=128))
        b_t = {}
        for nm, bd in [("b1", b1_d), ("b2", b2_d), ("b3", b3_d)]:
            t = const.tile([128, MT], F32, tag=nm)
            nc.sync.dma_start(out=t[:], in_=bd.rearrange("(m p) -> p m", p=128))
            b_t[nm] = t
        b4r_t = const.tile([128, P], F32, tag="b4r")
        nc.sync.dma_start(out=b4r_t[:], in_=b4r_d[:])
        msk_t = const.tile([128, 2, NN], F32, tag="msk")
        nc.sync.dma_start(out=msk_t[:], in_=msk_d.rearrange("(g p) n -> p g n", p=128))
        trow_t = const.tile([128, 2], F32, tag="trow")
        nc.sync.dma_start(out=trow_t[:], in_=trow_d.rearrange("(g p) -> p g", p=128))

        # ---- MLP layers 1-3 (output transposed: hidden on partitions) ----
        w2_r = w2_d.rearrange("(k p) (m c) -> p k m c", p=128, c=128)
        w3_r = w3_d.rearrange("(k p) (m c) -> p k m c", p=128, c=128)

        h1_t = hpool.tile([128, MT, RPC], dt_mm, tag="h")
        for m in range(MT):
            acc = ps.tile([128, RPC], F32, tag="acc")
            for k in range(KT1):
                nc.tensor.matmul(acc[:], w1_t[:, k, m * 128:(m + 1) * 128],
                                 xt_t[:, k, :], start=(k == 0), stop=(k == KT1 - 1))
            emit_lrelu(small, h1_t[:, m, :], acc[:], b_t["b1"][:, m:m + 1])

        for rep in range(l2_reps):
            h2_t = hpool.tile([128, MT, RPC], dt_mm, tag="h")
            for m in range(MT):
                ws = wstream.tile([128, KT, 128], dt_mm, tag="ws")
                nc.sync.dma_start(out=ws[:], in_=w2_r[:, :, m, :])
                acc = ps.tile([128, RPC], F32, tag="acc")
                for k in range(KT):
                    nc.tensor.matmul(acc[:], ws[:, k, :], h1_t[:, k, :],
                                     start=(k == 0), stop=(k == KT - 1))
                emit_lrelu(small, h2_t[:, m, :], acc[:], b_t["b2"][:, m:m + 1])

        h3_t = hpool.tile([128, MT, RPC], dt_mm, tag="h")
        for m in range(MT):
            ws = wstream.tile([128, KT, 128], dt_mm, tag="ws")
            nc.sync.dma_start(out=ws[:], in_=w3_r[:, :, m, :])
            acc = ps.tile([128, RPC], F32, tag="acc")
            for k in range(KT):
                nc.tensor.matmul(acc[:], ws[:, k, :], h2_t[:, k, :],
                                 start=(k == 0), stop=(k == KT - 1))
            emit_lrelu(small, h3_t[:, m, :], acc[:], b_t["b3"][:, m:m + 1])

        # ---- layer 4 (swapped operands: rows on partitions) + power iteration ----
        for g in range(RPC // 128):
            accA = ps.tile([128, P], F32, tag="acc")
            for k in range(KT):
                nc.tensor.matmul(accA[:], h3_t[:, k, g * 128:(g + 1) * 128],
                                 w4_t[:, k, :], start=(k == 0), stop=(k == KT - 1))
            pre = small.tile([128, P], F32, tag="pre")
            nc.vector.tensor_add(pre[:], accA[:], b4r_t[:])
            lr = small.tile([128, P], F32, tag="lr")
            emit_lrelu(small, lr[:], pre[:], 0.0)
            A_t = small.tile([128, P], F32, tag="A")
            nc.scalar.activation(A_t[:], lr[:], mybir.ActivationFunctionType.Sigmoid)
            A3 = A_t[:].rearrange("p (i j) -> p i j", j=NN)

            # power iteration: v0 = ones; per-step max-normalization is folded
            # into the next multiply (the final vec/v_src ratio is
            # scale-invariant, so any per-step scaling is valid).
            v_t = pipool.tile([128, NN], F32, tag="v")
            nc.vector.memset(v_t[:], 1.0)
            inv_t = pipool.tile([128, 1], F32, tag="inv")
            nc.vector.memset(inv_t[:], 1.0)
            for it in range(pi_iters):
                v_b = v_t[:].rearrange("p (a j) -> p a j", a=1).to_broadcast((128, NN, NN))
                prod = pipool.tile([128, NN, NN], F32, tag="prod")
                nc.vector.scalar_tensor_tensor(
                    prod[:], A3, inv_t[:, 0:1], v_b,
                    op0=mybir.AluOpType.mult, op1=mybir.AluOpType.mult)
                w_t = pipool.tile([128, NN], F32, tag="v")
                nc.vector.tensor_reduce(w_t[:], prod[:], axis=mybir.AxisListType.X,
                                        op=mybir.AluOpType.add)
                if it < pi_iters - 1:
                    mx = pipool.tile([128, 1], F32, tag="mx")
                    nc.vector.tensor_reduce(mx[:], w_t[:], axis=mybir.AxisListType.X,
                                            op=mybir.AluOpType.max)
                    inv_t = pipool.tile([128, 1], F32, tag="inv")
                    nc.vector.reciprocal(inv_t[:], mx[:])
                v_t = w_t

            # contrib = vec / vec[src] * T
            dummy = pipool.tile([128, NN], F32, tag="dummy")
            nc.vector.tensor_mul(dummy[:], v_t[:], msk_t[:, g, :])
            vsrc = pipool.tile([128, 1], F32, tag="vsrc")
            nc.vector.tensor_reduce(vsrc[:], dummy[:], axis=mybir.AxisListType.X,
                                    op=mybir.AluOpType.add)
            rsrc = pipool.tile([128, 1], F32, tag="rsrc")
            nc.vector.reciprocal(rsrc[:], vsrc[:])
            rt = pipool.tile([128, 1], F32, tag="rt")
            nc.vector.tensor_mul(rt[:], rsrc[:], trow_t[:, g:g + 1])
            contrib = pipool.tile([128, NN], F32, tag="contrib")
            nc.vector.tensor_scalar_mul(contrib[:], v_t[:], rt[:, 0:1])
            nc.sync.dma_start(out=out_d[g * 128:(g + 1) * 128, :], in_=contrib[:])

    nc.compile()
    return nc


def _np_dtype(dt_mm):
    import ml_dtypes
    if dt_mm == mybir.dt.bfloat16:
        return np.dtype(ml_dtypes.bfloat16)
    return np.float32


def _host_prep(inputs, dt_mm):
    """Build per-core input maps. Pure layout/indexing work."""
    ndt = _np_dtype(dt_mm)
    emb = np.asarray(inputs["nodes_embeddings"], dtype=np.float32)   # (B, N, D)
    Ts = np.asarray(inputs["Ts"], dtype=np.float32)                  # (B, N, N)
    W1 = np.ascontiguousarray(np.asarray(inputs["W1"], np.float32)).astype(ndt)
    W2 = np.ascontiguousarray(np.asarray(inputs["W2"], np.float32)).astype(ndt)
    W3 = np.ascontiguousarray(np.asarray(inputs["W3"], np.float32)).astype(ndt)
    W4 = np.ascontiguousarray(np.asarray(inputs["W4"], np.float32)).astype(ndt)
    b1 = np.ascontiguousarray(np.asarray(inputs["b1"], np.float32))
    b2 = np.ascontiguousarray(np.asarray(inputs["b2"], np.float32))
    b3 = np.ascontiguousarray(np.asarray(inputs["b3"], np.float32))
    b4 = np.asarray(inputs["b4"], np.float32)
    b4r = np.ascontiguousarray(np.broadcast_to(b4[None, :], (128, P)))

    embT = emb.transpose(1, 0, 2)                       # (N, B, D)
    pair = np.arange(P)
    s_ids = pair // NN
    t_ids = pair % NN
    # faithful to the reference: stack then flat-reinterpret, NOT per-row concat
    x = np.stack([embT[s_ids], embT[t_ids]], axis=1).reshape(P, B, 2 * D)
    xr = x.reshape(P * B, 2 * D)                        # (2048, 256)

    T_pair = Ts.reshape(B, P).T                         # (P, B)
    msk_full = (np.arange(NN)[None, :] == s_ids[:, None]).astype(np.float32)  # (P, NN)

    in_maps = []
    ppc = P // NCORES                                   # 32 pairs per core
    for c in range(NCORES):
        rows = slice(c * RPC, (c + 1) * RPC)
        prs = slice(c * ppc, (c + 1) * ppc)
        xt_c = np.ascontiguousarray(xr[rows].T).astype(ndt)          # (256, RPC)
        trow_c = np.ascontiguousarray(T_pair[prs, :].reshape(RPC))   # (RPC,)
        msk_c = np.ascontiguousarray(np.repeat(msk_full[prs], B, axis=0))  # (RPC, NN)
        in_maps.append({
            "xt": xt_c, "w1": W1, "w2": W2, "w3": W3, "w4": W4,
            "b1": b1, "b2": b2, "b3": b3, "b4r": b4r,
            "trow": trow_c, "msk": msk_c,
        })
    return in_maps


def _get_program(dt_mm, sim_safe=False, l2_reps=1):
    key = (str(dt_mm), sim_safe, l2_reps)
    if key not in _cache:
        _cache[key] = _build_program(dt_mm, sim_safe=sim_safe, l2_reps=l2_reps)
    return _cache[key]


def run(inputs, dt_mm=mybir.dt.float32r, trace=False):
    nc = _get_program(dt_mm)
    in_maps = _host_prep(inputs, dt_mm)
    res = run_bass_kernel_spmd(nc, in_maps, list(range(NCORES)), trace=trace)
    contribs = [res.results[c]["contrib"].astype(np.float32) for c in range(NCORES)]
    # unshard: sum pair contributions within each core block, then across cores
    out = np.zeros((B, NN), dtype=np.float64)
    for c in range(NCORES):
        out += contribs[c].astype(np.float64).reshape(P // NCORES, B, NN).sum(axis=0)
    return out.astype(np.float32), res


def kernel(**inputs):
    out, _ = run(inputs)
    return out


# revision 8
# speedup vs baseline: 8.9525x; 6.3193x over previous
"""Trainium2 Bass kernel for the pairwise-MLP + power-iteration module.

Computation (see host reference):
  - For each of P=256 (s,t) node pairs and B=8 graphs, build a 256-d feature row
    (a flat reinterpretation of stacked s/t embeddings), run a 256->4096->4096
    ->4096->256 LeakyReLU MLP with a final sigmoid -> a 16x16 positive matrix.
  - Power-iterate each matrix to its Perron eigenvector, divide by the source
    component, scale by Ts[pair], and sum contributions over all pairs -> (B, N).

Distribution: data-parallel over the P axis; each of the 8 cores runs 32 pairs
x 8 graphs = 256 MLP rows. Activations stay SBUF-resident in transposed layout
(hidden on partitions, rows on the free axis); W2/W3 stream from HBM in
column-stripes. Matmuls run in float32r (TF32-like multiply, fp32 accumulate).
The per-core [256 rows, 16] contribution block is returned and the final
pair/core reduction happens on the host (the all-reduce step).
"""

import numpy as np
from contextlib import ExitStack

import concourse.bass as bass
import concourse.tile as tile
from concourse import bacc, mybir
from concourse.bass_utils import run_bass_kernel_spmd

B = 8
NN = 16
D = 128
H = 4096
P = NN * NN            # 256 pairs
NCORES = 8
RPC = P * B // NCORES  # 256 rows per core
SLOPE = 0.01
PI_ITERS = 12

F32 = mybir.dt.float32

_cache = {}


def _build_program(dt_mm, pi_iters=PI_ITERS, sim_safe=False, l2_reps=1):
    KT1 = 2 * D // 128   # 2 k-tiles for layer 1
    KT = H // 128        # 32 k-tiles for layers 2-4
    MT = H // 128        # 32 m-stripes for layers 1-3

    nc = bacc.Bacc("TRN2", target_bir_lowering=False, debug=False,
                   num_devices=NCORES)

    def emit_lrelu(pool, out_ap, in_ap, bias):
        """out = LeakyReLU(in + bias). CoreSim lacks Lrelu, so the sim build
        decomposes it as pre*alpha + relu(pre)*(1-alpha)."""
        if not sim_safe:
            nc.scalar.activation(out_ap, in_ap, mybir.ActivationFunctionType.Lrelu,
                                 bias=bias, scale=1.0, alpha=SLOPE)
            return
        shape = [in_ap.shape[0], in_ap.free_size()]
        pre = pool.tile(shape, F32, tag="lr_pre")
        nc.scalar.activation(pre[:], in_ap, mybir.ActivationFunctionType.Identity,
                             bias=bias, scale=1.0)
        pos = pool.tile(shape, F32, tag="lr_pos")
        nc.scalar.activation(pos[:], pre[:], mybir.ActivationFunctionType.Relu)
        t1 = pool.tile(shape, F32, tag="lr_t1")
        nc.vector.tensor_scalar_mul(t1[:], pos[:], 1.0 - SLOPE)
        nc.vector.scalar_tensor_tensor(out_ap, pre[:], SLOPE, t1[:],
                                       op0=mybir.AluOpType.mult,
                                       op1=mybir.AluOpType.add)

    xt_d = nc.dram_tensor("xt", [2 * D, RPC], dt_mm, kind="ExternalInput").ap()
    w1_d = nc.dram_tensor("w1", [2 * D, H], dt_mm, kind="ExternalInput").ap()
    w2_d = nc.dram_tensor("w2", [H, H], dt_mm, kind="ExternalInput").ap()
    w3_d = nc.dram_tensor("w3", [H, H], dt_mm, kind="ExternalInput").ap()
    w4_d = nc.dram_tensor("w4", [H, P], dt_mm, kind="ExternalInput").ap()
    b1_d = nc.dram_tensor("b1", [H], F32, kind="ExternalInput").ap()
    b2_d = nc.dram_tensor("b2", [H], F32, kind="ExternalInput").ap()
    b3_d = nc.dram_tensor("b3", [H], F32, kind="ExternalInput").ap()
    b4r_d = nc.dram_tensor("b4r", [128, P], F32, kind="ExternalInput").ap()
    trow_d = nc.dram_tensor("trow", [RPC], F32, kind="ExternalInput").ap()
    msk_d = nc.dram_tensor("msk", [RPC, NN], F32, kind="ExternalInput").ap()
    out_d = nc.dram_tensor("contrib", [RPC, NN], F32, kind="ExternalOutput").ap()

    with tile.TileContext(nc) as tc, ExitStack() as ctx:
        const = ctx.enter_context(tc.tile_pool(name="const", bufs=1))
        hpool = ctx.enter_context(tc.tile_pool(name="h", bufs=2))
        wstream = ctx.enter_context(tc.tile_pool(name="wstream", bufs=3))
        small = ctx.enter_context(tc.tile_pool(name="small", bufs=2))
        pipool = ctx.enter_context(tc.tile_pool(name="pi", bufs=2))
        ps = ctx.enter_context(tc.tile_pool(name="ps", bufs=4, space="PSUM"))

        # ---- resident loads ----
        xt_t = const.tile([128, KT1, RPC], dt_mm, tag="xt")
        nc.sync.dma_start(out=xt_t[:], in_=xt_d.rearrange("(k p) r -> p k r", p=128))
        w1_t = const.tile([128, KT1, H], dt_mm, tag="w1")
        nc.sync.dma_start(out=w1_t[:], in_=w1_d.rearrange("(k p) h -> p k h", p=128))
        w4_t = const.tile([128, KT, P], dt_mm, tag="w4")
        nc.sync.dma_start(out=w4_t[:], in_=w4_d.rearrange("(k p) u -> p k u", p=128))
        b_t = {}
        for nm, bd in [("b1", b1_d), ("b2", b2_d), ("b3", b3_d)]:
            t = const.tile([128, MT], F32, tag=nm)
            nc.sync.dma_start(out=t[:], in_=bd.rearrange("(m p) -> p m", p=128))
            b_t[nm] = t
        b4r_t = const.tile([128, P], F32, tag="b4r")
        nc.sync.dma_start(out=b4r_t[:], in_=b4r_d[:])
        msk_t = const.tile([128, 2, NN], F32, tag="msk")
        nc.sync.dma_start(out=msk_t[:], in_=msk_d.rearrange("(g p) n -> p g n", p=128))
        trow_t = const.tile([128, 2], F32, tag="trow")
        nc.sync.dma_start(out=trow_t[:], in_=trow_d.rearrange("(g p) -> p g", p=128))

        # ---- MLP layers 1-3 (output transposed: hidden on partitions) ----
        w2_r = w2_d.rearrange("(k p) (m c) -> p k m c", p=128, c=128)
        w3_r = w3_d.rearrange("(k p) (m c) -> p k m c", p=128, c=128)

        h1_t = hpool.tile([128, MT, RPC], dt_mm, tag="h")
        for m in range(MT):
            acc = ps.tile([128, RPC], F32, tag="acc")
            for k in range(KT1):
                nc.tensor.matmul(acc[:], w1_t[:, k, m * 128:(m + 1) * 128],
                                 xt_t[:, k, :], start=(k == 0), stop=(k == KT1 - 1))
            emit_lrelu(small, h1_t[:, m, :], acc[:], b_t["b1"][:, m:m + 1])

        src_t = h1_t
        for rep in range(l2_reps):
            h2_t = hpool.tile([128, MT, RPC], dt_mm, tag="h")
            for m in range(MT):
                ws = wstream.tile([128, KT, 128], dt_mm, tag="ws")
                nc.sync.dma_start(out=ws[:], in_=w2_r[:, :, m, :])
                acc = ps.tile([128, RPC], F32, tag="acc")
                for k in range(KT):
                    nc.tensor.matmul(acc[:], ws[:, k, :], src_t[:, k, :],
                                     start=(k == 0), stop=(k == KT - 1))
                emit_lrelu(small, h2_t[:, m, :], acc[:], b_t["b2"][:, m:m + 1])
            src_t = h2_t

        h3_t = hpool.tile([128, MT, RPC], dt_mm, tag="h")
        for m in range(MT):
            ws = wstream.tile([128, KT, 128], dt_mm, tag="ws")
            nc.sync.dma_start(out=ws[:], in_=w3_r[:, :, m, :])
            acc = ps.tile([128, RPC], F32, tag="acc")
            for k in range(KT):
                nc.tensor.matmul(acc[:], ws[:, k, :], h2_t[:, k, :],
                                 start=(k == 0), stop=(k == KT - 1))
            emit_lrelu(small, h3_t[:, m, :], acc[:], b_t["b3"][:, m:m + 1])

        # ---- layer 4 (swapped operands: rows on partitions) + power iteration ----
        for g in range(RPC // 128):
            accA = ps.tile([128, P], F32, tag="acc")
            for k in range(KT):
                nc.tensor.matmul(accA[:], h3_t[:, k, g * 128:(g + 1) * 128],
                                 w4_t[:, k, :], start=(k == 0), stop=(k == KT - 1))
            pre = small.tile([128, P], F32, tag="pre")
            nc.vector.tensor_add(pre[:], accA[:], b4r_t[:])
            lr = small.tile([128, P], F32, tag="lr")
            emit_lrelu(small, lr[:], pre[:], 0.0)
            A_t = small.tile([128, P], F32, tag="A")
            nc.scalar.activation(A_t[:], lr[:], mybir.ActivationFunctionType.Sigmoid)
            A3 = A_t[:].rearrange("p (i j) -> p i j", j=NN)

            # power iteration: v0 = ones; per-step max-normalization is folded
            # into the next multiply (the final vec/v_src ratio is
            # scale-invariant, so any per-step scaling is valid).
            v_t = pipool.tile([128, NN], F32, tag="v")
            nc.vector.memset(v_t[:], 1.0)
            inv_t = pipool.tile([128, 1], F32, tag="inv")
            nc.vector.memset(inv_t[:], 1.0)
            for it in range(pi_iters):
                v_b = v_t[:].rearrange("p (a j) -> p a j", a=1).to_broadcast((128, NN, NN))
                prod = pipool.tile([128, NN, NN], F32, tag="prod")
                nc.vector.scalar_tensor_tensor(
                    prod[:], A3, inv_t[:, 0:1], v_b,
                    op0=mybir.AluOpType.mult, op1=mybir.AluOpType.mult)
                w_t = pipool.tile([128, NN], F32, tag="v")
                nc.vector.tensor_reduce(w_t[:], prod[:], axis=mybir.AxisListType.X,
                                        op=mybir.AluOpType.add)
                if it < pi_iters - 1:
                    mx = pipool.tile([128, 1], F32, tag="mx")
                    nc.vector.tensor_reduce(mx[:], w_t[:], axis=mybir.AxisListType.X,
                                            op=mybir.AluOpType.max)
                    inv_t = pipool.tile([128, 1], F32, tag="inv")
                    nc.vector.reciprocal(inv_t[:], mx[:])
                v_t = w_t

            # contrib = vec / vec[src] * T
            dummy = pipool.tile([128, NN], F32, tag="dummy")
            nc.vector.tensor_mul(dummy[:], v_t[:], msk_t[:, g, :])
            vsrc = pipool.tile([128, 1], F32, tag="vsrc")
            nc.vector.tensor_reduce(vsrc[:], dummy[:], axis=mybir.AxisListType.X,
                                    op=mybir.AluOpType.add)
            rsrc = pipool.tile([128, 1], F32, tag="rsrc")
            nc.vector.reciprocal(rsrc[:], vsrc[:])
            rt = pipool.tile([128, 1], F32, tag="rt")
            nc.vector.tensor_mul(rt[:], rsrc[:], trow_t[:, g:g + 1])
            contrib = pipool.tile([128, NN], F32, tag="contrib")
            nc.vector.tensor_scalar_mul(contrib[:], v_t[:], rt[:, 0:1])
            nc.sync.dma_start(out=out_d[g * 128:(g + 1) * 128, :], in_=contrib[:])

    nc.compile()
    return nc


def _np_dtype(dt_mm):
    import ml_dtypes
    if dt_mm == mybir.dt.bfloat16:
        return np.dtype(ml_dtypes.bfloat16)
    return np.float32


def _host_prep(inputs, dt_mm):
    """Build per-core input maps. Pure layout/indexing work."""
    ndt = _np_dtype(dt_mm)
    emb = np.asarray(inputs["nodes_embeddings"], dtype=np.float32)   # (B, N, D)
    Ts = np.asarray(inputs["Ts"], dtype=np.float32)                  # (B, N, N)
    W1 = np.ascontiguousarray(np.asarray(inputs["W1"], np.float32)).astype(ndt)
    W2 = np.ascontiguousarray(np.asarray(inputs["W2"], np.float32)).astype(ndt)
    W3 = np.ascontiguousarray(np.asarray(inputs["W3"], np.float32)).astype(ndt)
    W4 = np.ascontiguousarray(np.asarray(inputs["W4"], np.float32)).astype(ndt)
    b1 = np.ascontiguousarray(np.asarray(inputs["b1"], np.float32))
    b2 = np.ascontiguousarray(np.asarray(inputs["b2"], np.float32))
    b3 = np.ascontiguousarray(np.asarray(inputs["b3"], np.float32))
    b4 = np.asarray(inputs["b4"], np.float32)
    b4r = np.ascontiguousarray(np.broadcast_to(b4[None, :], (128, P)))

    embT = emb.transpose(1, 0, 2)                       # (N, B, D)
    pair = np.arange(P)
    s_ids = pair // NN
    t_ids = pair % NN
    # faithful to the reference: stack then flat-reinterpret, NOT per-row concat
    x = np.stack([embT[s_ids], embT[t_ids]], axis=1).reshape(P, B, 2 * D)
    xr = x.reshape(P * B, 2 * D)                        # (2048, 256)

    T_pair = Ts.reshape(B, P).T                         # (P, B)
    msk_full = (np.arange(NN)[None, :] == s_ids[:, None]).astype(np.float32)  # (P, NN)

    in_maps = []
    ppc = P // NCORES                                   # 32 pairs per core
    for c in range(NCORES):
        rows = slice(c * RPC, (c + 1) * RPC)
        prs = slice(c * ppc, (c + 1) * ppc)
        xt_c = np.ascontiguousarray(xr[rows].T).astype(ndt)          # (256, RPC)
        trow_c = np.ascontiguousarray(T_pair[prs, :].reshape(RPC))   # (RPC,)
        msk_c = np.ascontiguousarray(np.repeat(msk_full[prs], B, axis=0))  # (RPC, NN)
        in_maps.append({
            "xt": xt_c, "w1": W1, "w2": W2, "w3": W3, "w4": W4,
            "b1": b1, "b2": b2, "b3": b3, "b4r": b4r,
            "trow": trow_c, "msk": msk_c,
        })
    return in_maps


def _get_program(dt_mm, sim_safe=False, l2_reps=1):
    key = (str(dt_mm), sim_safe, l2_reps)
    if key not in _cache:
        _cache[key] = _build_program(dt_mm, sim_safe=sim_safe, l2_reps=l2_reps)
    return _cache[key]


def run(inputs, dt_mm=mybir.dt.float32r, trace=False):
    nc = _get_program(dt_mm)
    in_maps = _host_prep(inputs, dt_mm)
    res = run_bass_kernel_spmd(nc, in_maps, list(range(NCORES)), trace=trace)
    contribs = [res.results[c]["contrib"].astype(np.float32) for c in range(NCORES)]
    # unshard: sum pair contributions within each core block, then across cores
    out = np.zeros((B, NN), dtype=np.float64)
    for c in range(NCORES):
        out += contribs[c].astype(np.float64).reshape(P // NCORES, B, NN).sum(axis=0)
    return out.astype(np.float32), res


def kernel(**inputs):
    out, _ = run(inputs)
    return out


# revision 9
# speedup vs baseline: 14.6441x; 1.6358x over previous
"""Trainium2 Bass kernel for the pairwise-MLP + power-iteration module.

Computation (see host reference):
  - For each of P=256 (s,t) node pairs and B=8 graphs, build a 256-d feature row
    (a flat reinterpretation of stacked s/t embeddings), run a 256->4096->4096
    ->4096->256 LeakyReLU MLP with a final sigmoid -> a 16x16 positive matrix.
  - Power-iterate each matrix to its Perron eigenvector, divide by the source
    component, scale by Ts[pair], and sum contributions over all pairs -> (B, N).

Distribution: data-parallel over the P axis; each of the 8 cores runs 32 pairs
x 8 graphs = 256 MLP rows. Activations stay SBUF-resident in transposed layout
(hidden on partitions, rows on the free axis); W2/W3 stream from HBM in
column-stripes. Matmuls run in float32r (TF32-like multiply, fp32 accumulate).
The per-core [256 rows, 16] contribution block is returned and the final
pair/core reduction happens on the host (the all-reduce step).
"""

import numpy as np
from contextlib import ExitStack

import concourse.bass as bass
import concourse.tile as tile
from concourse import bacc, mybir
from concourse.bass_utils import run_bass_kernel_spmd

B = 8
NN = 16
D = 128
H = 4096
P = NN * NN            # 256 pairs
NCORES = 8
RPC = P * B // NCORES  # 256 rows per core
SLOPE = 0.01
PI_ITERS = 12

F32 = mybir.dt.float32

_cache = {}


def _build_program(dt_mm, pi_iters=PI_ITERS, sim_safe=False, l2_reps=1):
    KT1 = 2 * D // 128   # 2 k-tiles for layer 1
    KT = H // 128        # 32 k-tiles for layers 2-4
    MT = H // 128        # 32 m-stripes for layers 1-3
    MT_G, KT_G = MT, KT

    nc = bacc.Bacc("TRN2", target_bir_lowering=False, debug=False,
                   num_devices=NCORES)

    def emit_lrelu(pool, out_ap, in_ap, bias):
        """out = LeakyReLU(in + bias). CoreSim lacks Lrelu, so the sim build
        decomposes it as pre*alpha + relu(pre)*(1-alpha)."""
        if not sim_safe:
            nc.scalar.activation(out_ap, in_ap, mybir.ActivationFunctionType.Lrelu,
                                 bias=bias, scale=1.0, alpha=SLOPE)
            return
        shape = [in_ap.shape[0], in_ap.free_size()]
        pre = pool.tile(shape, F32, tag="lr_pre")
        nc.scalar.activation(pre[:], in_ap, mybir.ActivationFunctionType.Identity,
                             bias=bias, scale=1.0)
        pos = pool.tile(shape, F32, tag="lr_pos")
        nc.scalar.activation(pos[:], pre[:], mybir.ActivationFunctionType.Relu)
        t1 = pool.tile(shape, F32, tag="lr_t1")
        nc.vector.tensor_scalar_mul(t1[:], pos[:], 1.0 - SLOPE)
        nc.vector.scalar_tensor_tensor(out_ap, pre[:], SLOPE, t1[:],
                                       op0=mybir.AluOpType.mult,
                                       op1=mybir.AluOpType.add)

    xt_d = nc.dram_tensor("xt", [2 * D, RPC], dt_mm, kind="ExternalInput").ap()
    w1_d = nc.dram_tensor("w1", [2 * D, H], dt_mm, kind="ExternalInput").ap()
    # stripe-major host-rearranged layouts: one contiguous block per m-stripe,
    # 16KB+ contiguous per partition line (keeps DMA descriptors large)
    w2_d = nc.dram_tensor("w2", [MT_G, 128, KT_G, 128], dt_mm, kind="ExternalInput").ap()
    w3_d = nc.dram_tensor("w3", [MT_G, 128, KT_G, 128], dt_mm, kind="ExternalInput").ap()
    w4_d = nc.dram_tensor("w4", [128, KT_G, P], dt_mm, kind="ExternalInput").ap()
    b1_d = nc.dram_tensor("b1", [H], F32, kind="ExternalInput").ap()
    b2_d = nc.dram_tensor("b2", [H], F32, kind="ExternalInput").ap()
    b3_d = nc.dram_tensor("b3", [H], F32, kind="ExternalInput").ap()
    b4r_d = nc.dram_tensor("b4r", [128, P], F32, kind="ExternalInput").ap()
    trow_d = nc.dram_tensor("trow", [RPC], F32, kind="ExternalInput").ap()
    msk_d = nc.dram_tensor("msk", [RPC, NN], F32, kind="ExternalInput").ap()
    out_d = nc.dram_tensor("contrib", [RPC, NN], F32, kind="ExternalOutput").ap()

    with tile.TileContext(nc) as tc, ExitStack() as ctx:
        const = ctx.enter_context(tc.tile_pool(name="const", bufs=1))
        hpool = ctx.enter_context(tc.tile_pool(name="h", bufs=2))
        wstream = ctx.enter_context(tc.tile_pool(name="wstream", bufs=3))
        small = ctx.enter_context(tc.tile_pool(name="small", bufs=2))
        pipool = ctx.enter_context(tc.tile_pool(name="pi", bufs=2))
        ps = ctx.enter_context(tc.tile_pool(name="ps", bufs=4, space="PSUM"))

        # ---- resident loads ----
        xt_t = const.tile([128, KT1, RPC], dt_mm, tag="xt")
        nc.sync.dma_start(out=xt_t[:], in_=xt_d.rearrange("(k p) r -> p k r", p=128))
        w1_t = const.tile([128, KT1, H], dt_mm, tag="w1")
        nc.sync.dma_start(out=w1_t[:], in_=w1_d.rearrange("(k p) h -> p k h", p=128))
        w4_t = const.tile([128, KT, P], dt_mm, tag="w4")
        nc.sync.dma_start(out=w4_t[:], in_=w4_d[:])
        b_t = {}
        for nm, bd in [("b1", b1_d), ("b2", b2_d), ("b3", b3_d)]:
            t = const.tile([128, MT], F32, tag=nm)
            nc.sync.dma_start(out=t[:], in_=bd.rearrange("(m p) -> p m", p=128))
            b_t[nm] = t
        b4r_t = const.tile([128, P], F32, tag="b4r")
        nc.sync.dma_start(out=b4r_t[:], in_=b4r_d[:])
        msk_t = const.tile([128, 2, NN], F32, tag="msk")
        nc.sync.dma_start(out=msk_t[:], in_=msk_d.rearrange("(g p) n -> p g n", p=128))
        trow_t = const.tile([128, 2], F32, tag="trow")
        nc.sync.dma_start(out=trow_t[:], in_=trow_d.rearrange("(g p) -> p g", p=128))

        # ---- MLP layers 1-3 (output transposed: hidden on partitions) ----


        h1_t = hpool.tile([128, MT, RPC], dt_mm, tag="h")
        for m in range(MT):
            acc = ps.tile([128, RPC], F32, tag="acc")
            for k in range(KT1):
                nc.tensor.matmul(acc[:], w1_t[:, k, m * 128:(m + 1) * 128],
                                 xt_t[:, k, :], start=(k == 0), stop=(k == KT1 - 1))
            emit_lrelu(small, h1_t[:, m, :], acc[:], b_t["b1"][:, m:m + 1])

        src_t = h1_t
        for rep in range(l2_reps):
            h2_t = hpool.tile([128, MT, RPC], dt_mm, tag="h")
            for m in range(MT):
                ws = wstream.tile([128, KT, 128], dt_mm, tag="ws")
                nc.sync.dma_start(out=ws[:], in_=w2_d[m])
                acc = ps.tile([128, RPC], F32, tag="acc")
                for k in range(KT):
                    nc.tensor.matmul(acc[:], ws[:, k, :], src_t[:, k, :],
                                     start=(k == 0), stop=(k == KT - 1))
                emit_lrelu(small, h2_t[:, m, :], acc[:], b_t["b2"][:, m:m + 1])
            src_t = h2_t

        h3_t = hpool.tile([128, MT, RPC], dt_mm, tag="h")
        for m in range(MT):
            ws = wstream.tile([128, KT, 128], dt_mm, tag="ws")
            nc.sync.dma_start(out=ws[:], in_=w3_d[m])
            acc = ps.tile([128, RPC], F32, tag="acc")
            for k in range(KT):
                nc.tensor.matmul(acc[:], ws[:, k, :], h2_t[:, k, :],
                                 start=(k == 0), stop=(k == KT - 1))
            emit_lrelu(small, h3_t[:, m, :], acc[:], b_t["b3"][:, m:m + 1])

        # ---- layer 4 (swapped operands: rows on partitions) + power iteration ----
        for g in range(RPC // 128):
            accA = ps.tile([128, P], F32, tag="acc")
            for k in range(KT):
                nc.tensor.matmul(accA[:], h3_t[:, k, g * 128:(g + 1) * 128],
                                 w4_t[:, k, :], start=(k == 0), stop=(k == KT - 1))
            pre = small.tile([128, P], F32, tag="pre")
            nc.vector.tensor_add(pre[:], accA[:], b4r_t[:])
            lr = small.tile([128, P], F32, tag="lr")
            emit_lrelu(small, lr[:], pre[:], 0.0)
            A_t = small.tile([128, P], F32, tag="A")
            nc.scalar.activation(A_t[:], lr[:], mybir.ActivationFunctionType.Sigmoid)
            A3 = A_t[:].rearrange("p (i j) -> p i j", j=NN)

            # power iteration: v0 = ones; per-step max-normalization is folded
            # into the next multiply (the final vec/v_src ratio is
            # scale-invariant, so any per-step scaling is valid).
            v_t = pipool.tile([128, NN], F32, tag="v")
            nc.vector.memset(v_t[:], 1.0)
            inv_t = pipool.tile([128, 1], F32, tag="inv")
            nc.vector.memset(inv_t[:], 1.0)
            for it in range(pi_iters):
                v_b = v_t[:].rearrange("p (a j) -> p a j", a=1).to_broadcast((128, NN, NN))
                prod = pipool.tile([128, NN, NN], F32, tag="prod")
                nc.vector.scalar_tensor_tensor(
                    prod[:], A3, inv_t[:, 0:1], v_b,
                    op0=mybir.AluOpType.mult, op1=mybir.AluOpType.mult)
                w_t = pipool.tile([128, NN], F32, tag="v")
                nc.vector.tensor_reduce(w_t[:], prod[:], axis=mybir.AxisListType.X,
                                        op=mybir.AluOpType.add)
                if it < pi_iters - 1:
                    mx = pipool.tile([128, 1], F32, tag="mx")
                    nc.vector.tensor_reduce(mx[:], w_t[:], axis=mybir.AxisListType.X,
                                            op=mybir.AluOpType.max)
                    inv_t = pipool.tile([128, 1], F32, tag="inv")
                    nc.vector.reciprocal(inv_t[:], mx[:])
                v_t = w_t

            # contrib = vec / vec[src] * T
            dummy = pipool.tile([128, NN], F32, tag="dummy")
            nc.vector.tensor_mul(dummy[:], v_t[:], msk_t[:, g, :])
            vsrc = pipool.tile([128, 1], F32, tag="vsrc")
            nc.vector.tensor_reduce(vsrc[:], dummy[:], axis=mybir.AxisListType.X,
                                    op=mybir.AluOpType.add)
            rsrc = pipool.tile([128, 1], F32, tag="rsrc")
            nc.vector.reciprocal(rsrc[:], vsrc[:])
            rt = pipool.tile([128, 1], F32, tag="rt")
            nc.vector.tensor_mul(rt[:], rsrc[:], trow_t[:, g:g + 1])
            contrib = pipool.tile([128, NN], F32, tag="contrib")
            nc.vector.tensor_scalar_mul(contrib[:], v_t[:], rt[:, 0:1])
            nc.sync.dma_start(out=out_d[g * 128:(g + 1) * 128, :], in_=contrib[:])

    nc.compile()
    return nc


def _np_dtype(dt_mm):
    import ml_dtypes
    if dt_mm == mybir.dt.bfloat16:
        return np.dtype(ml_dtypes.bfloat16)
    return np.float32


def _host_prep(inputs, dt_mm):
    """Build per-core input maps. Pure layout/indexing work."""
    ndt = _np_dtype(dt_mm)
    emb = np.asarray(inputs["nodes_embeddings"], dtype=np.float32)   # (B, N, D)
    Ts = np.asarray(inputs["Ts"], dtype=np.float32)                  # (B, N, N)
    W1 = np.ascontiguousarray(np.asarray(inputs["W1"], np.float32)).astype(ndt)
    MT = KT = H // 128

    def stripe_layout(w):  # [H, H] -> [m, p, k, c] contiguous
        return np.ascontiguousarray(
            np.asarray(w, np.float32).reshape(KT, 128, MT, 128).transpose(2, 1, 0, 3)
        ).astype(ndt)

    W2 = stripe_layout(inputs["W2"])
    W3 = stripe_layout(inputs["W3"])
    W4 = np.ascontiguousarray(
        np.asarray(inputs["W4"], np.float32).reshape(KT, 128, P).transpose(1, 0, 2)
    ).astype(ndt)
    b1 = np.ascontiguousarray(np.asarray(inputs["b1"], np.float32))
    b2 = np.ascontiguousarray(np.asarray(inputs["b2"], np.float32))
    b3 = np.ascontiguousarray(np.asarray(inputs["b3"], np.float32))
    b4 = np.asarray(inputs["b4"], np.float32)
    b4r = np.ascontiguousarray(np.broadcast_to(b4[None, :], (128, P)))

    embT = emb.transpose(1, 0, 2)                       # (N, B, D)
    pair = np.arange(P)
    s_ids = pair // NN
    t_ids = pair % NN
    # faithful to the reference: stack then flat-reinterpret, NOT per-row concat
    x = np.stack([embT[s_ids], embT[t_ids]], axis=1).reshape(P, B, 2 * D)
    xr = x.reshape(P * B, 2 * D)                        # (2048, 256)

    T_pair = Ts.reshape(B, P).T                         # (P, B)
    msk_full = (np.arange(NN)[None, :] == s_ids[:, None]).astype(np.float32)  # (P, NN)

    in_maps = []
    ppc = P // NCORES                                   # 32 pairs per core
    for c in range(NCORES):
        rows = slice(c * RPC, (c + 1) * RPC)
        prs = slice(c * ppc, (c + 1) * ppc)
        xt_c = np.ascontiguousarray(xr[rows].T).astype(ndt)          # (256, RPC)
        trow_c = np.ascontiguousarray(T_pair[prs, :].reshape(RPC))   # (RPC,)
        msk_c = np.ascontiguousarray(np.repeat(msk_full[prs], B, axis=0))  # (RPC, NN)
        in_maps.append({
            "xt": xt_c, "w1": W1, "w2": W2, "w3": W3, "w4": W4,
            "b1": b1, "b2": b2, "b3": b3, "b4r": b4r,
            "trow": trow_c, "msk": msk_c,
        })
    return in_maps


def _get_program(dt_mm, sim_safe=False, l2_reps=1):
    key = (str(dt_mm), sim_safe, l2_reps)
    if key not in _cache:
        _cache[key] = _build_program(dt_mm, sim_safe=sim_safe, l2_reps=l2_reps)
    return _cache[key]


def run(inputs, dt_mm=mybir.dt.float32r, trace=False):
    nc = _get_program(dt_mm)
    in_maps = _host_prep(inputs, dt_mm)
    res = run_bass_kernel_spmd(nc, in_maps, list(range(NCORES)), trace=trace)
    contribs = [res.results[c]["contrib"].astype(np.float32) for c in range(NCORES)]
    # unshard: sum pair contributions within each core block, then across cores
    out = np.zeros((B, NN), dtype=np.float64)
    for c in range(NCORES):
        out += contribs[c].astype(np.float64).reshape(P // NCORES, B, NN).sum(axis=0)
    return out.astype(np.float32), res


def kernel(**inputs):
    out, _ = run(inputs)
    return out


# revision 10
# speedup vs baseline: 22.3527x; 1.5264x over previous
"""Trainium2 Bass kernel for the pairwise-MLP + power-iteration module.

Computation (see host reference):
  - For each of P=256 (s,t) node pairs and B=8 graphs, build a 256-d feature row
    (a flat reinterpretation of stacked s/t embeddings), run a 256->4096->4096
    ->4096->256 LeakyReLU MLP with a final sigmoid -> a 16x16 positive matrix.
  - Power-iterate each matrix to its Perron eigenvector, divide by the source
    component, scale by Ts[pair], and sum contributions over all pairs -> (B, N).

Distribution: data-parallel over the P axis; each of the 8 cores runs 32 pairs
x 8 graphs = 256 MLP rows. Activations stay SBUF-resident in transposed layout
(hidden on partitions, rows on the free axis); W2/W3 stream from HBM in
column-stripes. Matmuls run in float32r (TF32-like multiply, fp32 accumulate).
The per-core [256 rows, 16] contribution block is returned and the final
pair/core reduction happens on the host (the all-reduce step).
"""

import numpy as np
from contextlib import ExitStack

import concourse.bass as bass
import concourse.tile as tile
from concourse import bacc, mybir
from concourse.bass_utils import run_bass_kernel_spmd

B = 8
NN = 16
D = 128
H = 4096
P = NN * NN            # 256 pairs
NCORES = 8
RPC = P * B // NCORES  # 256 rows per core
SLOPE = 0.01
PI_ITERS = 12

F32 = mybir.dt.float32

_cache = {}


def _build_program(dt_mm, pi_iters=PI_ITERS, sim_safe=False, l2_reps=1):
    KT1 = 2 * D // 128   # 2 k-tiles for layer 1
    KT = H // 128        # 32 k-tiles for layers 2-4
    MT = H // 128        # 32 m-stripes for layers 1-3
    MT_G, KT_G = MT, KT

    nc = bacc.Bacc("TRN2", target_bir_lowering=False, debug=False,
                   num_devices=NCORES)

    def emit_lrelu(pool, out_ap, in_ap, bias):
        """out = LeakyReLU(in + bias). CoreSim lacks Lrelu, so the sim build
        decomposes it as pre*alpha + relu(pre)*(1-alpha)."""
        if not sim_safe:
            nc.scalar.activation(out_ap, in_ap, mybir.ActivationFunctionType.Lrelu,
                                 bias=bias, scale=1.0, alpha=SLOPE)
            return
        shape = [in_ap.shape[0], in_ap.free_size()]
        pre = pool.tile(shape, F32, tag="lr_pre")
        nc.scalar.activation(pre[:], in_ap, mybir.ActivationFunctionType.Identity,
                             bias=bias, scale=1.0)
        pos = pool.tile(shape, F32, tag="lr_pos")
        nc.scalar.activation(pos[:], pre[:], mybir.ActivationFunctionType.Relu)
        t1 = pool.tile(shape, F32, tag="lr_t1")
        nc.vector.tensor_scalar_mul(t1[:], pos[:], 1.0 - SLOPE)
        nc.vector.scalar_tensor_tensor(out_ap, pre[:], SLOPE, t1[:],
                                       op0=mybir.AluOpType.mult,
                                       op1=mybir.AluOpType.add)

    xt_d = nc.dram_tensor("xt", [2 * D, RPC], dt_mm, kind="ExternalInput").ap()
    w1_d = nc.dram_tensor("w1", [2 * D, H], dt_mm, kind="ExternalInput").ap()
    # stripe-major host-rearranged layouts: one contiguous block per m-stripe,
    # 16KB+ contiguous per partition line (keeps DMA descriptors large)
    w2_d = nc.dram_tensor("w2", [MT_G, 128, KT_G, 128], dt_mm, kind="ExternalInput").ap()
    w3_d = nc.dram_tensor("w3", [MT_G, 128, KT_G, 128], dt_mm, kind="ExternalInput").ap()
    w4_d = nc.dram_tensor("w4", [128, KT_G, P], dt_mm, kind="ExternalInput").ap()
    b1_d = nc.dram_tensor("b1", [H], F32, kind="ExternalInput").ap()
    b2_d = nc.dram_tensor("b2", [H], F32, kind="ExternalInput").ap()
    b3_d = nc.dram_tensor("b3", [H], F32, kind="ExternalInput").ap()
    b4r_d = nc.dram_tensor("b4r", [128, P], F32, kind="ExternalInput").ap()
    trow_d = nc.dram_tensor("trow", [RPC], F32, kind="ExternalInput").ap()
    msk_d = nc.dram_tensor("msk", [RPC, NN], F32, kind="ExternalInput").ap()
    out_d = nc.dram_tensor("contrib", [RPC, NN], F32, kind="ExternalOutput").ap()

    with tile.TileContext(nc) as tc, ExitStack() as ctx:
        esz = 2 if dt_mm in (mybir.dt.float16, mybir.dt.bfloat16) else 4
        ws_bufs = 5 if esz == 2 else 3
        const = ctx.enter_context(tc.tile_pool(name="const", bufs=1))
        hpool = ctx.enter_context(tc.tile_pool(name="h", bufs=2))
        wstream = ctx.enter_context(tc.tile_pool(name="wstream", bufs=ws_bufs))
        small = ctx.enter_context(tc.tile_pool(name="small", bufs=2))
        pipool = ctx.enter_context(tc.tile_pool(name="pi", bufs=2))
        ps = ctx.enter_context(tc.tile_pool(name="ps", bufs=6, space="PSUM"))

        # ---- resident loads ----
        xt_t = const.tile([128, KT1, RPC], dt_mm, tag="xt")
        nc.sync.dma_start(out=xt_t[:], in_=xt_d.rearrange("(k p) r -> p k r", p=128))
        w1_t = const.tile([128, KT1, H], dt_mm, tag="w1")
        nc.sync.dma_start(out=w1_t[:], in_=w1_d.rearrange("(k p) h -> p k h", p=128))
        w4_t = const.tile([128, KT, P], dt_mm, tag="w4")
        nc.sync.dma_start(out=w4_t[:], in_=w4_d[:])
        b_t = {}
        for nm, bd in [("b1", b1_d), ("b2", b2_d), ("b3", b3_d)]:
            t = const.tile([128, MT], F32, tag=nm)
            nc.sync.dma_start(out=t[:], in_=bd.rearrange("(m p) -> p m", p=128))
            b_t[nm] = t
        b4r_t = const.tile([128, P], F32, tag="b4r")
        nc.sync.dma_start(out=b4r_t[:], in_=b4r_d[:])
        msk_t = const.tile([128, 2, NN], F32, tag="msk")
        nc.sync.dma_start(out=msk_t[:], in_=msk_d.rearrange("(g p) n -> p g n", p=128))
        trow_t = const.tile([128, 2], F32, tag="trow")
        nc.sync.dma_start(out=trow_t[:], in_=trow_d.rearrange("(g p) -> p g", p=128))

        # ---- MLP layers 1-3 (output transposed: hidden on partitions) ----


        h1_t = hpool.tile([128, MT, RPC], dt_mm, tag="h")
        for m in range(MT):
            acc = ps.tile([128, RPC], F32, tag="acc")
            for k in range(KT1):
                nc.tensor.matmul(acc[:], w1_t[:, k, m * 128:(m + 1) * 128],
                                 xt_t[:, k, :], start=(k == 0), stop=(k == KT1 - 1))
            emit_lrelu(small, h1_t[:, m, :], acc[:], b_t["b1"][:, m:m + 1])

        src_t = h1_t
        for rep in range(l2_reps):
            h2_t = hpool.tile([128, MT, RPC], dt_mm, tag="h")
            for m in range(MT):
                ws = wstream.tile([128, KT, 128], dt_mm, tag="ws")
                nc.sync.dma_start(out=ws[:], in_=w2_d[m])
                acc = ps.tile([128, RPC], F32, tag="acc")
                for k in range(KT):
                    nc.tensor.matmul(acc[:], ws[:, k, :], src_t[:, k, :],
                                     start=(k == 0), stop=(k == KT - 1))
                emit_lrelu(small, h2_t[:, m, :], acc[:], b_t["b2"][:, m:m + 1])
            src_t = h2_t

        h3_t = hpool.tile([128, MT, RPC], dt_mm, tag="h")
        for m in range(MT):
            ws = wstream.tile([128, KT, 128], dt_mm, tag="ws")
            nc.sync.dma_start(out=ws[:], in_=w3_d[m])
            acc = ps.tile([128, RPC], F32, tag="acc")
            for k in range(KT):
                nc.tensor.matmul(acc[:], ws[:, k, :], h2_t[:, k, :],
                                 start=(k == 0), stop=(k == KT - 1))
            emit_lrelu(small, h3_t[:, m, :], acc[:], b_t["b3"][:, m:m + 1])

        # ---- layer 4 (swapped operands: rows on partitions) + power iteration ----
        for g in range(RPC // 128):
            accA = ps.tile([128, P], F32, tag="acc")
            for k in range(KT):
                nc.tensor.matmul(accA[:], h3_t[:, k, g * 128:(g + 1) * 128],
                                 w4_t[:, k, :], start=(k == 0), stop=(k == KT - 1))
            pre = small.tile([128, P], F32, tag="pre")
            nc.vector.tensor_add(pre[:], accA[:], b4r_t[:])
            lr = small.tile([128, P], F32, tag="lr")
            emit_lrelu(small, lr[:], pre[:], 0.0)
            A_t = small.tile([128, P], F32, tag="A")
            nc.scalar.activation(A_t[:], lr[:], mybir.ActivationFunctionType.Sigmoid)
            A3 = A_t[:].rearrange("p (i j) -> p i j", j=NN)

            # power iteration: v0 = ones; per-step max-normalization is folded
            # into the next multiply (the final vec/v_src ratio is
            # scale-invariant, so any per-step scaling is valid).
            v_t = pipool.tile([128, NN], F32, tag="v")
            nc.vector.memset(v_t[:], 1.0)
            inv_t = pipool.tile([128, 1], F32, tag="inv")
            nc.vector.memset(inv_t[:], 1.0)
            for it in range(pi_iters):
                v_b = v_t[:].rearrange("p (a j) -> p a j", a=1).to_broadcast((128, NN, NN))
                prod = pipool.tile([128, NN, NN], F32, tag="prod")
                nc.vector.scalar_tensor_tensor(
                    prod[:], A3, inv_t[:, 0:1], v_b,
                    op0=mybir.AluOpType.mult, op1=mybir.AluOpType.mult)
                w_t = pipool.tile([128, NN], F32, tag="v")
                nc.vector.tensor_reduce(w_t[:], prod[:], axis=mybir.AxisListType.X,
                                        op=mybir.AluOpType.add)
                if it < pi_iters - 1:
                    mx = pipool.tile([128, 1], F32, tag="mx")
                    nc.vector.tensor_reduce(mx[:], w_t[:], axis=mybir.AxisListType.X,
                                            op=mybir.AluOpType.max)
                    inv_t = pipool.tile([128, 1], F32, tag="inv")
                    nc.vector.reciprocal(inv_t[:], mx[:])
                v_t = w_t

            # contrib = vec / vec[src] * T
            dummy = pipool.tile([128, NN], F32, tag="dummy")
            nc.vector.tensor_mul(dummy[:], v_t[:], msk_t[:, g, :])
            vsrc = pipool.tile([128, 1], F32, tag="vsrc")
            nc.vector.tensor_reduce(vsrc[:], dummy[:], axis=mybir.AxisListType.X,
                                    op=mybir.AluOpType.add)
            rsrc = pipool.tile([128, 1], F32, tag="rsrc")
            nc.vector.reciprocal(rsrc[:], vsrc[:])
            rt = pipool.tile([128, 1], F32, tag="rt")
            nc.vector.tensor_mul(rt[:], rsrc[:], trow_t[:, g:g + 1])
            contrib = pipool.tile([128, NN], F32, tag="contrib")
            nc.vector.tensor_scalar_mul(contrib[:], v_t[:], rt[:, 0:1])
            nc.sync.dma_start(out=out_d[g * 128:(g + 1) * 128, :], in_=contrib[:])

    nc.compile()
    return nc


def _np_dtype(dt_mm):
    import ml_dtypes
    if dt_mm == mybir.dt.bfloat16:
        return np.dtype(ml_dtypes.bfloat16)
    if dt_mm == mybir.dt.float16:
        return np.dtype(np.float16)
    return np.float32


def _host_prep(inputs, dt_mm):
    """Build per-core input maps. Pure layout/indexing work."""
    ndt = _np_dtype(dt_mm)
    emb = np.asarray(inputs["nodes_embeddings"], dtype=np.float32)   # (B, N, D)
    Ts = np.asarray(inputs["Ts"], dtype=np.float32)                  # (B, N, N)
    W1 = np.ascontiguousarray(np.asarray(inputs["W1"], np.float32)).astype(ndt)
    MT = KT = H // 128

    def stripe_layout(w):  # [H, H] -> [m, p, k, c] contiguous
        return np.ascontiguousarray(
            np.asarray(w, np.float32).reshape(KT, 128, MT, 128).transpose(2, 1, 0, 3)
        ).astype(ndt)

    W2 = stripe_layout(inputs["W2"])
    W3 = stripe_layout(inputs["W3"])
    W4 = np.ascontiguousarray(
        np.asarray(inputs["W4"], np.float32).reshape(KT, 128, P).transpose(1, 0, 2)
    ).astype(ndt)
    b1 = np.ascontiguousarray(np.asarray(inputs["b1"], np.float32))
    b2 = np.ascontiguousarray(np.asarray(inputs["b2"], np.float32))
    b3 = np.ascontiguousarray(np.asarray(inputs["b3"], np.float32))
    b4 = np.asarray(inputs["b4"], np.float32)
    b4r = np.ascontiguousarray(np.broadcast_to(b4[None, :], (128, P)))

    embT = emb.transpose(1, 0, 2)                       # (N, B, D)
    pair = np.arange(P)
    s_ids = pair // NN
    t_ids = pair % NN
    # faithful to the reference: stack then flat-reinterpret, NOT per-row concat
    x = np.stack([embT[s_ids], embT[t_ids]], axis=1).reshape(P, B, 2 * D)
    xr = x.reshape(P * B, 2 * D)                        # (2048, 256)

    T_pair = Ts.reshape(B, P).T                         # (P, B)
    msk_full = (np.arange(NN)[None, :] == s_ids[:, None]).astype(np.float32)  # (P, NN)

    in_maps = []
    ppc = P // NCORES                                   # 32 pairs per core
    for c in range(NCORES):
        rows = slice(c * RPC, (c + 1) * RPC)
        prs = slice(c * ppc, (c + 1) * ppc)
        xt_c = np.ascontiguousarray(xr[rows].T).astype(ndt)          # (256, RPC)
        trow_c = np.ascontiguousarray(T_pair[prs, :].reshape(RPC))   # (RPC,)
        msk_c = np.ascontiguousarray(np.repeat(msk_full[prs], B, axis=0))  # (RPC, NN)
        in_maps.append({
            "xt": xt_c, "w1": W1, "w2": W2, "w3": W3, "w4": W4,
            "b1": b1, "b2": b2, "b3": b3, "b4r": b4r,
            "trow": trow_c, "msk": msk_c,
        })
    return in_maps


def _get_program(dt_mm, sim_safe=False, l2_reps=1):
    key = (str(dt_mm), sim_safe, l2_reps)
    if key not in _cache:
        _cache[key] = _build_program(dt_mm, sim_safe=sim_safe, l2_reps=l2_reps)
    return _cache[key]


def run(inputs, dt_mm=mybir.dt.float32r, trace=False):
    nc = _get_program(dt_mm)
    in_maps = _host_prep(inputs, dt_mm)
    res = run_bass_kernel_spmd(nc, in_maps, list(range(NCORES)), trace=trace)
    contribs = [res.results[c]["contrib"].astype(np.float32) for c in range(NCORES)]
    # unshard: sum pair contributions within each core block, then across cores
    out = np.zeros((B, NN), dtype=np.float64)
    for c in range(NCORES):
        out += contribs[c].astype(np.float64).reshape(P // NCORES, B, NN).sum(axis=0)
    return out.astype(np.float32), res


def kernel(**inputs):
    out, _ = run(inputs)
    return out


# revision 11
# speedup vs baseline: 22.7727x; 1.0188x over previous
"""Trainium2 Bass kernel for the pairwise-MLP + power-iteration module.

Computation (see host reference):
  - For each of P=256 (s,t) node pairs and B=8 graphs, build a 256-d feature row
    (a flat reinterpretation of stacked s/t embeddings), run a 256->4096->4096
    ->4096->256 LeakyReLU MLP with a final sigmoid -> a 16x16 positive matrix.
  - Power-iterate each matrix to its Perron eigenvector, divide by the source
    component, scale by Ts[pair], and sum contributions over all pairs -> (B, N).

Distribution: data-parallel over the P axis; each of the 8 cores runs 32 pairs
x 8 graphs = 256 MLP rows. Activations stay SBUF-resident in transposed layout
(hidden on partitions, rows on the free axis); W2/W3 stream from HBM in
column-stripes. Matmuls run in float32r (TF32-like multiply, fp32 accumulate).
The per-core [256 rows, 16] contribution block is returned and the final
pair/core reduction happens on the host (the all-reduce step).
"""

import numpy as np
from contextlib import ExitStack

import concourse.bass as bass
import concourse.tile as tile
from concourse import bacc, mybir
from concourse.bass_utils import run_bass_kernel_spmd

B = 8
NN = 16
D = 128
H = 4096
P = NN * NN            # 256 pairs
NCORES = 8
RPC = P * B // NCORES  # 256 rows per core
SLOPE = 0.01
PI_ITERS = 10

F32 = mybir.dt.float32

_cache = {}


def _build_program(dt_mm, pi_iters=PI_ITERS, sim_safe=False, l2_reps=1):
    KT1 = 2 * D // 128   # 2 k-tiles for layer 1
    KT = H // 128        # 32 k-tiles for layers 2-4
    MT = H // 128        # 32 m-stripes for layers 1-3
    MT_G, KT_G = MT, KT

    nc = bacc.Bacc("TRN2", target_bir_lowering=False, debug=False,
                   num_devices=NCORES)

    def emit_lrelu(pool, out_ap, in_ap, bias):
        """out = LeakyReLU(in + bias). CoreSim lacks Lrelu, so the sim build
        decomposes it as pre*alpha + relu(pre)*(1-alpha)."""
        if not sim_safe:
            nc.scalar.activation(out_ap, in_ap, mybir.ActivationFunctionType.Lrelu,
                                 bias=bias, scale=1.0, alpha=SLOPE)
            return
        shape = [in_ap.shape[0], in_ap.free_size()]
        pre = pool.tile(shape, F32, tag="lr_pre")
        nc.scalar.activation(pre[:], in_ap, mybir.ActivationFunctionType.Identity,
                             bias=bias, scale=1.0)
        pos = pool.tile(shape, F32, tag="lr_pos")
        nc.scalar.activation(pos[:], pre[:], mybir.ActivationFunctionType.Relu)
        t1 = pool.tile(shape, F32, tag="lr_t1")
        nc.vector.tensor_scalar_mul(t1[:], pos[:], 1.0 - SLOPE)
        nc.vector.scalar_tensor_tensor(out_ap, pre[:], SLOPE, t1[:],
                                       op0=mybir.AluOpType.mult,
                                       op1=mybir.AluOpType.add)

    xt_d = nc.dram_tensor("xt", [2 * D, RPC], dt_mm, kind="ExternalInput").ap()
    w1_d = nc.dram_tensor("w1", [2 * D, H], dt_mm, kind="ExternalInput").ap()
    # stripe-major host-rearranged layouts: one contiguous block per m-stripe,
    # 16KB+ contiguous per partition line (keeps DMA descriptors large)
    w2_d = nc.dram_tensor("w2", [MT_G, 128, KT_G, 128], dt_mm, kind="ExternalInput").ap()
    w3_d = nc.dram_tensor("w3", [MT_G, 128, KT_G, 128], dt_mm, kind="ExternalInput").ap()
    w4_d = nc.dram_tensor("w4", [128, KT_G, P], dt_mm, kind="ExternalInput").ap()
    b1_d = nc.dram_tensor("b1", [H], F32, kind="ExternalInput").ap()
    b2_d = nc.dram_tensor("b2", [H], F32, kind="ExternalInput").ap()
    b3_d = nc.dram_tensor("b3", [H], F32, kind="ExternalInput").ap()
    b4r_d = nc.dram_tensor("b4r", [128, P], F32, kind="ExternalInput").ap()
    trow_d = nc.dram_tensor("trow", [RPC], F32, kind="ExternalInput").ap()
    msk_d = nc.dram_tensor("msk", [RPC, NN], F32, kind="ExternalInput").ap()
    out_d = nc.dram_tensor("contrib", [RPC, NN], F32, kind="ExternalOutput").ap()

    with tile.TileContext(nc) as tc, ExitStack() as ctx:
        esz = 2 if dt_mm in (mybir.dt.float16, mybir.dt.bfloat16) else 4
        ws_bufs = 3 if esz == 2 else 2
        const = ctx.enter_context(tc.tile_pool(name="const", bufs=1))
        hpool = ctx.enter_context(tc.tile_pool(name="h", bufs=2))
        wstream = ctx.enter_context(tc.tile_pool(name="wstream", bufs=ws_bufs))
        small = ctx.enter_context(tc.tile_pool(name="small", bufs=2))
        pipool = ctx.enter_context(tc.tile_pool(name="pi", bufs=2))
        ps = ctx.enter_context(tc.tile_pool(name="ps", bufs=6, space="PSUM"))

        # ---- resident loads ----
        xt_t = const.tile([128, KT1, RPC], dt_mm, tag="xt")
        nc.sync.dma_start(out=xt_t[:], in_=xt_d.rearrange("(k p) r -> p k r", p=128))
        w1_t = const.tile([128, KT1, H], dt_mm, tag="w1")
        nc.sync.dma_start(out=w1_t[:], in_=w1_d.rearrange("(k p) h -> p k h", p=128))
        w4_t = const.tile([128, KT, P], dt_mm, tag="w4")
        nc.sync.dma_start(out=w4_t[:], in_=w4_d[:])
        b_t = {}
        for nm, bd in [("b1", b1_d), ("b2", b2_d), ("b3", b3_d)]:
            t = const.tile([128, MT], F32, tag=nm)
            nc.sync.dma_start(out=t[:], in_=bd.rearrange("(m p) -> p m", p=128))
            b_t[nm] = t
        b4r_t = const.tile([128, P], F32, tag="b4r")
        nc.sync.dma_start(out=b4r_t[:], in_=b4r_d[:])
        msk_t = const.tile([128, 2, NN], F32, tag="msk")
        nc.sync.dma_start(out=msk_t[:], in_=msk_d.rearrange("(g p) n -> p g n", p=128))
        trow_t = const.tile([128, 2], F32, tag="trow")
        nc.sync.dma_start(out=trow_t[:], in_=trow_d.rearrange("(g p) -> p g", p=128))

        # ---- MLP layers 1-3 (output transposed: hidden on partitions) ----


        h1_t = hpool.tile([128, MT, RPC], dt_mm, tag="h")
        for m in range(MT):
            acc = ps.tile([128, RPC], F32, tag="acc")
            for k in range(KT1):
                nc.tensor.matmul(acc[:], w1_t[:, k, m * 128:(m + 1) * 128],
                                 xt_t[:, k, :], start=(k == 0), stop=(k == KT1 - 1))
            emit_lrelu(small, h1_t[:, m, :], acc[:], b_t["b1"][:, m:m + 1])

        src_t = h1_t
        for rep in range(l2_reps):
            h2_t = hpool.tile([128, MT, RPC], dt_mm, tag="h")
            for m0 in range(0, MT, 2):
                ws = wstream.tile([128, 2, KT, 128], dt_mm, tag="ws")
                nc.sync.dma_start(out=ws[:], in_=w2_d[m0:m0 + 2].rearrange("m p k c -> p m k c"))
                for mi in range(2):
                    m = m0 + mi
                    acc = ps.tile([128, RPC], F32, tag="acc")
                    for k in range(KT):
                        nc.tensor.matmul(acc[:], ws[:, mi, k, :], src_t[:, k, :],
                                         start=(k == 0), stop=(k == KT - 1))
                    emit_lrelu(small, h2_t[:, m, :], acc[:], b_t["b2"][:, m:m + 1])
            src_t = h2_t

        h3_t = hpool.tile([128, MT, RPC], dt_mm, tag="h")
        for m0 in range(0, MT, 2):
            ws = wstream.tile([128, 2, KT, 128], dt_mm, tag="ws")
            nc.sync.dma_start(out=ws[:], in_=w3_d[m0:m0 + 2].rearrange("m p k c -> p m k c"))
            for mi in range(2):
                m = m0 + mi
                acc = ps.tile([128, RPC], F32, tag="acc")
                for k in range(KT):
                    nc.tensor.matmul(acc[:], ws[:, mi, k, :], h2_t[:, k, :],
                                     start=(k == 0), stop=(k == KT - 1))
                emit_lrelu(small, h3_t[:, m, :], acc[:], b_t["b3"][:, m:m + 1])

        # ---- layer 4 (swapped operands: rows on partitions) + power iteration ----
        for g in range(RPC // 128):
            accA = ps.tile([128, P], F32, tag="acc")
            for k in range(KT):
                nc.tensor.matmul(accA[:], h3_t[:, k, g * 128:(g + 1) * 128],
                                 w4_t[:, k, :], start=(k == 0), stop=(k == KT - 1))
            pre = small.tile([128, P], F32, tag="pre")
            nc.vector.tensor_add(pre[:], accA[:], b4r_t[:])
            lr = small.tile([128, P], F32, tag="lr")
            emit_lrelu(small, lr[:], pre[:], 0.0)
            A_t = small.tile([128, P], F32, tag="A")
            nc.scalar.activation(A_t[:], lr[:], mybir.ActivationFunctionType.Sigmoid)
            A3 = A_t[:].rearrange("p (i j) -> p i j", j=NN)

            # power iteration: v0 = ones; per-step max-normalization is folded
            # into the next multiply (the final vec/v_src ratio is
            # scale-invariant, so any per-step scaling is valid).
            v_t = pipool.tile([128, NN], F32, tag="v")
            nc.vector.memset(v_t[:], 1.0)
            inv_t = pipool.tile([128, 1], F32, tag="inv")
            nc.vector.memset(inv_t[:], 1.0)
            for it in range(pi_iters):
                v_b = v_t[:].rearrange("p (a j) -> p a j", a=1).to_broadcast((128, NN, NN))
                prod = pipool.tile([128, NN, NN], F32, tag="prod")
                nc.vector.scalar_tensor_tensor(
                    prod[:], A3, inv_t[:, 0:1], v_b,
                    op0=mybir.AluOpType.mult, op1=mybir.AluOpType.mult)
                w_t = pipool.tile([128, NN], F32, tag="v")
                nc.vector.tensor_reduce(w_t[:], prod[:], axis=mybir.AxisListType.X,
                                        op=mybir.AluOpType.add)
                if it < pi_iters - 1:
                    mx = pipool.tile([128, 1], F32, tag="mx")
                    nc.vector.tensor_reduce(mx[:], w_t[:], axis=mybir.AxisListType.X,
                                            op=mybir.AluOpType.max)
                    inv_t = pipool.tile([128, 1], F32, tag="inv")
                    nc.vector.reciprocal(inv_t[:], mx[:])
                v_t = w_t

            # contrib = vec / vec[src] * T
            dummy = pipool.tile([128, NN], F32, tag="dummy")
            nc.vector.tensor_mul(dummy[:], v_t[:], msk_t[:, g, :])
            vsrc = pipool.tile([128, 1], F32, tag="vsrc")
            nc.vector.tensor_reduce(vsrc[:], dummy[:], axis=mybir.AxisListType.X,
                                    op=mybir.AluOpType.add)
            rsrc = pipool.tile([128, 1], F32, tag="rsrc")
            nc.vector.reciprocal(rsrc[:], vsrc[:])
            rt = pipool.tile([128, 1], F32, tag="rt")
            nc.vector.tensor_mul(rt[:], rsrc[:], trow_t[:, g:g + 1])
            contrib = pipool.tile([128, NN], F32, tag="contrib")
            nc.vector.tensor_scalar_mul(contrib[:], v_t[:], rt[:, 0:1])
            nc.sync.dma_start(out=out_d[g * 128:(g + 1) * 128, :], in_=contrib[:])

    nc.compile()
    return nc


def _np_dtype(dt_mm):
    import ml_dtypes
    if dt_mm == mybir.dt.bfloat16:
        return np.dtype(ml_dtypes.bfloat16)
    if dt_mm == mybir.dt.float16:
        return np.dtype(np.float16)
    return np.float32


def _host_prep(inputs, dt_mm):
    """Build per-core input maps. Pure layout/indexing work."""
    ndt = _np_dtype(dt_mm)
    emb = np.asarray(inputs["nodes_embeddings"], dtype=np.float32)   # (B, N, D)
    Ts = np.asarray(inputs["Ts"], dtype=np.float32)                  # (B, N, N)
    W1 = np.ascontiguousarray(np.asarray(inputs["W1"], np.float32)).astype(ndt)
    MT = KT = H // 128

    def stripe_layout(w):  # [H, H] -> [m, p, k, c] contiguous
        return np.ascontiguousarray(
            np.asarray(w, np.float32).reshape(KT, 128, MT, 128).transpose(2, 1, 0, 3)
        ).astype(ndt)

    W2 = stripe_layout(inputs["W2"])
    W3 = stripe_layout(inputs["W3"])
    W4 = np.ascontiguousarray(
        np.asarray(inputs["W4"], np.float32).reshape(KT, 128, P).transpose(1, 0, 2)
    ).astype(ndt)
    b1 = np.ascontiguousarray(np.asarray(inputs["b1"], np.float32))
    b2 = np.ascontiguousarray(np.asarray(inputs["b2"], np.float32))
    b3 = np.ascontiguousarray(np.asarray(inputs["b3"], np.float32))
    b4 = np.asarray(inputs["b4"], np.float32)
    b4r = np.ascontiguousarray(np.broadcast_to(b4[None, :], (128, P)))

    embT = emb.transpose(1, 0, 2)                       # (N, B, D)
    pair = np.arange(P)
    s_ids = pair // NN
    t_ids = pair % NN
    # faithful to the reference: stack then flat-reinterpret, NOT per-row concat
    x = np.stack([embT[s_ids], embT[t_ids]], axis=1).reshape(P, B, 2 * D)
    xr = x.reshape(P * B, 2 * D)                        # (2048, 256)

    T_pair = Ts.reshape(B, P).T                         # (P, B)
    msk_full = (np.arange(NN)[None, :] == s_ids[:, None]).astype(np.float32)  # (P, NN)

    in_maps = []
    ppc = P // NCORES                                   # 32 pairs per core
    for c in range(NCORES):
        rows = slice(c * RPC, (c + 1) * RPC)
        prs = slice(c * ppc, (c + 1) * ppc)
        xt_c = np.ascontiguousarray(xr[rows].T).astype(ndt)          # (256, RPC)
        trow_c = np.ascontiguousarray(T_pair[prs, :].reshape(RPC))   # (RPC,)
        msk_c = np.ascontiguousarray(np.repeat(msk_full[prs], B, axis=0))  # (RPC, NN)
        in_maps.append({
            "xt": xt_c, "w1": W1, "w2": W2, "w3": W3, "w4": W4,
            "b1": b1, "b2": b2, "b3": b3, "b4r": b4r,
            "trow": trow_c, "msk": msk_c,
        })
    return in_maps


def _get_program(dt_mm, sim_safe=False, l2_reps=1):
    key = (str(dt_mm), sim_safe, l2_reps)
    if key not in _cache:
        _cache[key] = _build_program(dt_mm, sim_safe=sim_safe, l2_reps=l2_reps)
    return _cache[key]


def run(inputs, dt_mm=mybir.dt.float32r, trace=False):
    nc = _get_program(dt_mm)
    in_maps = _host_prep(inputs, dt_mm)
    res = run_bass_kernel_spmd(nc, in_maps, list(range(NCORES)), trace=trace)
    contribs = [res.results[c]["contrib"].astype(np.float32) for c in range(NCORES)]
    # unshard: sum pair contributions within each core block, then across cores
    out = np.zeros((B, NN), dtype=np.float64)
    for c in range(NCORES):
        out += contribs[c].astype(np.float64).reshape(P // NCORES, B, NN).sum(axis=0)
    return out.astype(np.float32), res


def kernel(**inputs):
    out, _ = run(inputs)
    return out


# revision 13
# speedup vs baseline: 23.0698x; 1.0130x over previous
"""Trainium2 Bass kernel for the pairwise-MLP + power-iteration module.

Computation (see host reference):
  - For each of P=256 (s,t) node pairs and B=8 graphs, build a 256-d feature row
    (a flat reinterpretation of stacked s/t embeddings), run a 256->4096->4096
    ->4096->256 LeakyReLU MLP with a final sigmoid -> a 16x16 positive matrix.
  - Power-iterate each matrix to its Perron eigenvector, divide by the source
    component, scale by Ts[pair], and sum contributions over all pairs -> (B, N).

Distribution: data-parallel over the P axis; each of the 8 cores runs 32 pairs
x 8 graphs = 256 MLP rows. Activations stay SBUF-resident in transposed layout
(hidden on partitions, rows on the free axis); W2/W3 stream from HBM in
column-stripes. Matmuls run in float32r (TF32-like multiply, fp32 accumulate).
The per-core [256 rows, 16] contribution block is returned and the final
pair/core reduction happens on the host (the all-reduce step).
"""

import numpy as np
from contextlib import ExitStack

import concourse.bass as bass
import concourse.tile as tile
from concourse import bacc, mybir
from concourse.bass_utils import run_bass_kernel_spmd

B = 8
NN = 16
D = 128
H = 4096
P = NN * NN            # 256 pairs
NCORES = 8
RPC = P * B // NCORES  # 256 rows per core
SLOPE = 0.01
PI_ITERS = 10

F32 = mybir.dt.float32

_cache = {}


def _build_program(dt_mm, pi_iters=PI_ITERS, sim_safe=False, l2_reps=1):
    KT1 = 2 * D // 128   # 2 k-tiles for layer 1
    KT = H // 128        # 32 k-tiles for layers 2-4
    MT = H // 128        # 32 m-stripes for layers 1-3
    MT_G, KT_G = MT, KT

    nc = bacc.Bacc("TRN2", target_bir_lowering=False, debug=False,
                   num_devices=NCORES)

    def emit_lrelu(pool, out_ap, in_ap, bias):
        """out = LeakyReLU(in + bias). CoreSim lacks Lrelu, so the sim build
        decomposes it as pre*alpha + relu(pre)*(1-alpha)."""
        if not sim_safe:
            nc.scalar.activation(out_ap, in_ap, mybir.ActivationFunctionType.Lrelu,
                                 bias=bias, scale=1.0, alpha=SLOPE)
            return
        shape = [in_ap.shape[0], in_ap.free_size()]
        pre = pool.tile(shape, F32, tag="lr_pre")
        nc.scalar.activation(pre[:], in_ap, mybir.ActivationFunctionType.Identity,
                             bias=bias, scale=1.0)
        pos = pool.tile(shape, F32, tag="lr_pos")
        nc.scalar.activation(pos[:], pre[:], mybir.ActivationFunctionType.Relu)
        t1 = pool.tile(shape, F32, tag="lr_t1")
        nc.vector.tensor_scalar_mul(t1[:], pos[:], 1.0 - SLOPE)
        nc.vector.scalar_tensor_tensor(out_ap, pre[:], SLOPE, t1[:],
                                       op0=mybir.AluOpType.mult,
                                       op1=mybir.AluOpType.add)

    xt_d = nc.dram_tensor("xt", [2 * D, RPC], dt_mm, kind="ExternalInput").ap()
    w1_d = nc.dram_tensor("w1", [2 * D, H], dt_mm, kind="ExternalInput").ap()
    # stripe-major host-rearranged layouts: one contiguous block per m-stripe,
    # 16KB+ contiguous per partition line (keeps DMA descriptors large)
    w2_d = nc.dram_tensor("w2", [MT_G, 128, KT_G, 128], dt_mm, kind="ExternalInput").ap()
    w3_d = nc.dram_tensor("w3", [MT_G, 128, KT_G, 128], dt_mm, kind="ExternalInput").ap()
    w4_d = nc.dram_tensor("w4", [128, KT_G, P], dt_mm, kind="ExternalInput").ap()
    b1_d = nc.dram_tensor("b1", [H], F32, kind="ExternalInput").ap()
    b2_d = nc.dram_tensor("b2", [H], F32, kind="ExternalInput").ap()
    b3_d = nc.dram_tensor("b3", [H], F32, kind="ExternalInput").ap()
    b4r_d = nc.dram_tensor("b4r", [128, P], F32, kind="ExternalInput").ap()
    trow_d = nc.dram_tensor("trow", [RPC], F32, kind="ExternalInput").ap()
    msk_d = nc.dram_tensor("msk", [RPC, NN], F32, kind="ExternalInput").ap()
    out_d = nc.dram_tensor("contrib", [RPC, NN], F32, kind="ExternalOutput").ap()

    with tile.TileContext(nc) as tc, ExitStack() as ctx:
        esz = 2 if dt_mm in (mybir.dt.float16, mybir.dt.bfloat16) else 4
        ws_bufs = 3 if esz == 2 else 2
        const = ctx.enter_context(tc.tile_pool(name="const", bufs=1))
        hpool = ctx.enter_context(tc.tile_pool(name="h", bufs=2))
        wstream = ctx.enter_context(tc.tile_pool(name="wstream", bufs=ws_bufs))
        small = ctx.enter_context(tc.tile_pool(name="small", bufs=2))
        pipool = ctx.enter_context(tc.tile_pool(name="pi", bufs=2))
        ps = ctx.enter_context(tc.tile_pool(name="ps", bufs=6, space="PSUM"))

        # ---- resident loads ----
        xt_t = const.tile([128, KT1, RPC], dt_mm, tag="xt")
        nc.sync.dma_start(out=xt_t[:], in_=xt_d.rearrange("(k p) r -> p k r", p=128))
        w1_t = const.tile([128, KT1, H], dt_mm, tag="w1")
        nc.sync.dma_start(out=w1_t[:], in_=w1_d.rearrange("(k p) h -> p k h", p=128))
        b_t = {}
        for nm, bd in [("b1", b1_d), ("b2", b2_d), ("b3", b3_d)]:
            t = const.tile([128, MT], F32, tag=nm)
            nc.sync.dma_start(out=t[:], in_=bd.rearrange("(m p) -> p m", p=128))
            b_t[nm] = t

        # ---- MLP layers 1-3 (output transposed: hidden on partitions) ----


        h1_t = hpool.tile([128, MT, RPC], dt_mm, tag="h")
        for m in range(MT):
            acc = ps.tile([128, RPC], F32, tag="acc")
            for k in range(KT1):
                nc.tensor.matmul(acc[:], w1_t[:, k, m * 128:(m + 1) * 128],
                                 xt_t[:, k, :], start=(k == 0), stop=(k == KT1 - 1))
            emit_lrelu(small, h1_t[:, m, :], acc[:], b_t["b1"][:, m:m + 1])

        src_t = h1_t
        for rep in range(l2_reps):
            h2_t = hpool.tile([128, MT, RPC], dt_mm, tag="h")
            for m0 in range(0, MT, 2):
                ws = wstream.tile([128, 2, KT, 128], dt_mm, tag="ws")
                nc.sync.dma_start(out=ws[:], in_=w2_d[m0:m0 + 2].rearrange("m p k c -> p m k c"))
                for mi in range(2):
                    m = m0 + mi
                    acc = ps.tile([128, RPC], F32, tag="acc")
                    for k in range(KT):
                        nc.tensor.matmul(acc[:], ws[:, mi, k, :], src_t[:, k, :],
                                         start=(k == 0), stop=(k == KT - 1))
                    emit_lrelu(small, h2_t[:, m, :], acc[:], b_t["b2"][:, m:m + 1])
            src_t = h2_t

        h3_t = hpool.tile([128, MT, RPC], dt_mm, tag="h")
        for m0 in range(0, MT, 2):
            ws = wstream.tile([128, 2, KT, 128], dt_mm, tag="ws")
            nc.sync.dma_start(out=ws[:], in_=w3_d[m0:m0 + 2].rearrange("m p k c -> p m k c"))
            for mi in range(2):
                m = m0 + mi
                acc = ps.tile([128, RPC], F32, tag="acc")
                for k in range(KT):
                    nc.tensor.matmul(acc[:], ws[:, mi, k, :], h2_t[:, k, :],
                                     start=(k == 0), stop=(k == KT - 1))
                emit_lrelu(small, h3_t[:, m, :], acc[:], b_t["b3"][:, m:m + 1])

        # ---- layer 4 (swapped operands: rows on partitions) + power iteration ----
        # L4-only constants load late so their DMAs don't delay the L1/L2 weights
        w4_t = const.tile([128, KT, P], dt_mm, tag="w4")
        nc.sync.dma_start(out=w4_t[:], in_=w4_d[:])
        b4r_t = const.tile([128, P], F32, tag="b4r")
        nc.sync.dma_start(out=b4r_t[:], in_=b4r_d[:])
        msk_t = const.tile([128, 2, NN], F32, tag="msk")
        nc.sync.dma_start(out=msk_t[:], in_=msk_d.rearrange("(g p) n -> p g n", p=128))
        trow_t = const.tile([128, 2], F32, tag="trow")
        nc.sync.dma_start(out=trow_t[:], in_=trow_d.rearrange("(g p) -> p g", p=128))
        for g in range(RPC // 128):
            accA = ps.tile([128, P], F32, tag="acc")
            for k in range(KT):
                nc.tensor.matmul(accA[:], h3_t[:, k, g * 128:(g + 1) * 128],
                                 w4_t[:, k, :], start=(k == 0), stop=(k == KT - 1))
            pre = small.tile([128, P], F32, tag="pre")
            nc.vector.tensor_add(pre[:], accA[:], b4r_t[:])
            lr = small.tile([128, P], F32, tag="lr")
            emit_lrelu(small, lr[:], pre[:], 0.0)
            A_t = small.tile([128, P], F32, tag="A")
            nc.scalar.activation(A_t[:], lr[:], mybir.ActivationFunctionType.Sigmoid)
            A3 = A_t[:].rearrange("p (i j) -> p i j", j=NN)

            # power iteration: v0 = ones; per-step max-normalization is folded
            # into the next multiply (the final vec/v_src ratio is
            # scale-invariant, so any per-step scaling is valid).
            v_t = pipool.tile([128, NN], F32, tag="v")
            nc.vector.memset(v_t[:], 1.0)
            ones_t = pipool.tile([128, 1], F32, tag="ones")
            nc.vector.memset(ones_t[:], 1.0)
            inv_t = ones_t
            for it in range(pi_iters):
                v_b = v_t[:].rearrange("p (a j) -> p a j", a=1).to_broadcast((128, NN, NN))
                prod = pipool.tile([128, NN, NN], F32, tag="prod")
                nc.vector.scalar_tensor_tensor(
                    prod[:], A3, inv_t[:, 0:1], v_b,
                    op0=mybir.AluOpType.mult, op1=mybir.AluOpType.mult)
                w_t = pipool.tile([128, NN], F32, tag="v")
                nc.vector.tensor_reduce(w_t[:], prod[:], axis=mybir.AxisListType.X,
                                        op=mybir.AluOpType.add)
                if it < pi_iters - 1 and it % 2 == 1:
                    mx = pipool.tile([128, 1], F32, tag="mx")
                    nc.vector.tensor_reduce(mx[:], w_t[:], axis=mybir.AxisListType.X,
                                            op=mybir.AluOpType.max)
                    inv_t = pipool.tile([128, 1], F32, tag="inv")
                    nc.vector.reciprocal(inv_t[:], mx[:])
                else:
                    inv_t = ones_t
                v_t = w_t

            # contrib = vec / vec[src] * T
            dummy = pipool.tile([128, NN], F32, tag="dummy")
            nc.vector.tensor_mul(dummy[:], v_t[:], msk_t[:, g, :])
            vsrc = pipool.tile([128, 1], F32, tag="vsrc")
            nc.vector.tensor_reduce(vsrc[:], dummy[:], axis=mybir.AxisListType.X,
                                    op=mybir.AluOpType.add)
            rsrc = pipool.tile([128, 1], F32, tag="rsrc")
            nc.vector.reciprocal(rsrc[:], vsrc[:])
            rt = pipool.tile([128, 1], F32, tag="rt")
            nc.vector.tensor_mul(rt[:], rsrc[:], trow_t[:, g:g + 1])
            contrib = pipool.tile([128, NN], F32, tag="contrib")
            nc.vector.tensor_scalar_mul(contrib[:], v_t[:], rt[:, 0:1])
            nc.sync.dma_start(out=out_d[g * 128:(g + 1) * 128, :], in_=contrib[:])

    nc.compile()
    return nc


def _np_dtype(dt_mm):
    import ml_dtypes
    if dt_mm == mybir.dt.bfloat16:
        return np.dtype(ml_dtypes.bfloat16)
    if dt_mm == mybir.dt.float16:
        return np.dtype(np.float16)
    return np.float32


def _host_prep(inputs, dt_mm):
    """Build per-core input maps. Pure layout/indexing work."""
    ndt = _np_dtype(dt_mm)
    emb = np.asarray(inputs["nodes_embeddings"], dtype=np.float32)   # (B, N, D)
    Ts = np.asarray(inputs["Ts"], dtype=np.float32)                  # (B, N, N)
    W1 = np.ascontiguousarray(np.asarray(inputs["W1"], np.float32)).astype(ndt)
    MT = KT = H // 128

    def stripe_layout(w):  # [H, H] -> [m, p, k, c] contiguous
        return np.ascontiguousarray(
            np.asarray(w, np.float32).reshape(KT, 128, MT, 128).transpose(2, 1, 0, 3)
        ).astype(ndt)

    W2 = stripe_layout(inputs["W2"])
    W3 = stripe_layout(inputs["W3"])
    W4 = np.ascontiguousarray(
        np.asarray(inputs["W4"], np.float32).reshape(KT, 128, P).transpose(1, 0, 2)
    ).astype(ndt)
    b1 = np.ascontiguousarray(np.asarray(inputs["b1"], np.float32))
    b2 = np.ascontiguousarray(np.asarray(inputs["b2"], np.float32))
    b3 = np.ascontiguousarray(np.asarray(inputs["b3"], np.float32))
    b4 = np.asarray(inputs["b4"], np.float32)
    b4r = np.ascontiguousarray(np.broadcast_to(b4[None, :], (128, P)))

    embT = emb.transpose(1, 0, 2)                       # (N, B, D)
    pair = np.arange(P)
    s_ids = pair // NN
    t_ids = pair % NN
    # faithful to the reference: stack then flat-reinterpret, NOT per-row concat
    x = np.stack([embT[s_ids], embT[t_ids]], axis=1).reshape(P, B, 2 * D)
    xr = x.reshape(P * B, 2 * D)                        # (2048, 256)

    T_pair = Ts.reshape(B, P).T                         # (P, B)
    msk_full = (np.arange(NN)[None, :] == s_ids[:, None]).astype(np.float32)  # (P, NN)

    in_maps = []
    ppc = P // NCORES                                   # 32 pairs per core
    for c in range(NCORES):
        rows = slice(c * RPC, (c + 1) * RPC)
        prs = slice(c * ppc, (c + 1) * ppc)
        xt_c = np.ascontiguousarray(xr[rows].T).astype(ndt)          # (256, RPC)
        trow_c = np.ascontiguousarray(T_pair[prs, :].reshape(RPC))   # (RPC,)
        msk_c = np.ascontiguousarray(np.repeat(msk_full[prs], B, axis=0))  # (RPC, NN)
        in_maps.append({
            "xt": xt_c, "w1": W1, "w2": W2, "w3": W3, "w4": W4,
            "b1": b1, "b2": b2, "b3": b3, "b4r": b4r,
            "trow": trow_c, "msk": msk_c,
        })
    return in_maps


def _get_program(dt_mm, sim_safe=False, l2_reps=1):
    key = (str(dt_mm), sim_safe, l2_reps)
    if key not in _cache:
        _cache[key] = _build_program(dt_mm, sim_safe=sim_safe, l2_reps=l2_reps)
    return _cache[key]


def run(inputs, dt_mm=mybir.dt.float32r, trace=False):
    nc = _get_program(dt_mm)
    in_maps = _host_prep(inputs, dt_mm)
    res = run_bass_kernel_spmd(nc, in_maps, list(range(NCORES)), trace=trace)
    contribs = [res.results[c]["contrib"].astype(np.float32) for c in range(NCORES)]
    # unshard: sum pair contributions within each core block, then across cores
    out = np.zeros((B, NN), dtype=np.float64)
    for c in range(NCORES):
        out += contribs[c].astype(np.float64).reshape(P // NCORES, B, NN).sum(axis=0)
    return out.astype(np.float32), res


def kernel(**inputs):
    out, _ = run(inputs)
    return out


# revision 14
# speedup vs baseline: 24.1766x; 1.0480x over previous
"""Trainium2 Bass kernel for the pairwise-MLP + power-iteration module.

Computation (see host reference):
  - For each of P=256 (s,t) node pairs and B=8 graphs, build a 256-d feature row
    (a flat reinterpretation of stacked s/t embeddings), run a 256->4096->4096
    ->4096->256 LeakyReLU MLP with a final sigmoid -> a 16x16 positive matrix.
  - Power-iterate each matrix to its Perron eigenvector, divide by the source
    component, scale by Ts[pair], and sum contributions over all pairs -> (B, N).

Distribution: data-parallel over the P axis; each of the 8 cores runs 32 pairs
x 8 graphs = 256 MLP rows. Activations stay SBUF-resident in transposed layout
(hidden on partitions, rows on the free axis); W2/W3 stream from HBM in
column-stripes. Matmuls run in float32r (TF32-like multiply, fp32 accumulate).
The per-core [256 rows, 16] contribution block is returned and the final
pair/core reduction happens on the host (the all-reduce step).
"""

import numpy as np
from contextlib import ExitStack

import concourse.bass as bass
import concourse.tile as tile
from concourse import bacc, mybir
from concourse.bass_utils import run_bass_kernel_spmd

B = 8
NN = 16
D = 128
H = 4096
P = NN * NN            # 256 pairs
NCORES = 8
RPC = P * B // NCORES  # 256 rows per core
SLOPE = 0.01
PI_ITERS = 8

F32 = mybir.dt.float32

_cache = {}


def _build_program(dt_mm, pi_iters=PI_ITERS, sim_safe=False, l2_reps=1):
    KT1 = 2 * D // 128   # 2 k-tiles for layer 1
    KT = H // 128        # 32 k-tiles for layers 2-4
    MT = H // 128        # 32 m-stripes for layers 1-3
    MT_G, KT_G = MT, KT
    KT1_G = 2 * D // 128

    nc = bacc.Bacc("TRN2", target_bir_lowering=False, debug=False,
                   num_devices=NCORES)

    def emit_lrelu(pool, out_ap, in_ap, bias):
        """out = LeakyReLU(in + bias). CoreSim lacks Lrelu, so the sim build
        decomposes it as pre*alpha + relu(pre)*(1-alpha)."""
        if not sim_safe:
            nc.scalar.activation(out_ap, in_ap, mybir.ActivationFunctionType.Lrelu,
                                 bias=bias, scale=1.0, alpha=SLOPE)
            return
        shape = [in_ap.shape[0], in_ap.free_size()]
        pre = pool.tile(shape, F32, tag="lr_pre")
        nc.scalar.activation(pre[:], in_ap, mybir.ActivationFunctionType.Identity,
                             bias=bias, scale=1.0)
        pos = pool.tile(shape, F32, tag="lr_pos")
        nc.scalar.activation(pos[:], pre[:], mybir.ActivationFunctionType.Relu)
        t1 = pool.tile(shape, F32, tag="lr_t1")
        nc.vector.tensor_scalar_mul(t1[:], pos[:], 1.0 - SLOPE)
        nc.vector.scalar_tensor_tensor(out_ap, pre[:], SLOPE, t1[:],
                                       op0=mybir.AluOpType.mult,
                                       op1=mybir.AluOpType.add)

    xt_d = nc.dram_tensor("xt", [2 * D, RPC], dt_mm, kind="ExternalInput").ap()
    w1_d = nc.dram_tensor("w1", [4, 128, KT1_G, 8 * 128], dt_mm, kind="ExternalInput").ap()
    # stripe-major host-rearranged layouts: one contiguous block per m-stripe,
    # 16KB+ contiguous per partition line (keeps DMA descriptors large)
    w2_d = nc.dram_tensor("w2", [MT_G, 128, KT_G, 128], dt_mm, kind="ExternalInput").ap()
    w3_d = nc.dram_tensor("w3", [MT_G, 128, KT_G, 128], dt_mm, kind="ExternalInput").ap()
    w4_d = nc.dram_tensor("w4", [128, KT_G, P], dt_mm, kind="ExternalInput").ap()
    b1_d = nc.dram_tensor("b1", [H], F32, kind="ExternalInput").ap()
    b2_d = nc.dram_tensor("b2", [H], F32, kind="ExternalInput").ap()
    b3_d = nc.dram_tensor("b3", [H], F32, kind="ExternalInput").ap()
    b4r_d = nc.dram_tensor("b4r", [128, P], F32, kind="ExternalInput").ap()
    trow_d = nc.dram_tensor("trow", [RPC], F32, kind="ExternalInput").ap()
    msk_d = nc.dram_tensor("msk", [RPC, NN], F32, kind="ExternalInput").ap()
    out_d = nc.dram_tensor("contrib", [RPC, NN], F32, kind="ExternalOutput").ap()

    with tile.TileContext(nc) as tc, ExitStack() as ctx:
        esz = 2 if dt_mm in (mybir.dt.float16, mybir.dt.bfloat16) else 4
        ws_bufs = 3 if esz == 2 else 2
        const = ctx.enter_context(tc.tile_pool(name="const", bufs=1))
        hpool = ctx.enter_context(tc.tile_pool(name="h", bufs=2))
        wstream = ctx.enter_context(tc.tile_pool(name="wstream", bufs=ws_bufs))
        small = ctx.enter_context(tc.tile_pool(name="small", bufs=2))
        pipool = ctx.enter_context(tc.tile_pool(name="pi", bufs=2))
        ps = ctx.enter_context(tc.tile_pool(name="ps", bufs=6, space="PSUM"))

        # ---- resident loads ----
        xt_t = const.tile([128, KT1, RPC], dt_mm, tag="xt")
        nc.sync.dma_start(out=xt_t[:], in_=xt_d.rearrange("(k p) r -> p k r", p=128))
        b_t = {}
        for nm, bd in [("b1", b1_d), ("b2", b2_d), ("b3", b3_d)]:
            t = const.tile([128, MT], F32, tag=nm)
            nc.sync.dma_start(out=t[:], in_=bd.rearrange("(m p) -> p m", p=128))
            b_t[nm] = t

        # ---- MLP layers 1-3 (output transposed: hidden on partitions) ----


        h1_t = hpool.tile([128, MT, RPC], dt_mm, tag="h")
        w1_chunks = []
        for ch in range(4):
            w1c = const.tile([128, KT1, 8 * 128], dt_mm, tag=f"w1c{ch}")
            nc.sync.dma_start(out=w1c[:], in_=w1_d[ch])
            w1_chunks.append(w1c)
        for m in range(MT):
            w1c = w1_chunks[m // 8]
            mi = m % 8
            acc = ps.tile([128, RPC], F32, tag="acc")
            for k in range(KT1):
                nc.tensor.matmul(acc[:], w1c[:, k, mi * 128:(mi + 1) * 128],
                                 xt_t[:, k, :], start=(k == 0), stop=(k == KT1 - 1))
            emit_lrelu(small, h1_t[:, m, :], acc[:], b_t["b1"][:, m:m + 1])

        src_t = h1_t
        for rep in range(l2_reps):
            h2_t = hpool.tile([128, MT, RPC], dt_mm, tag="h")
            for m0 in range(0, MT, 2):
                ws = wstream.tile([128, 2, KT, 128], dt_mm, tag="ws")
                nc.sync.dma_start(out=ws[:], in_=w2_d[m0:m0 + 2].rearrange("m p k c -> p m k c"))
                for mi in range(2):
                    m = m0 + mi
                    acc = ps.tile([128, RPC], F32, tag="acc")
                    for k in range(KT):
                        nc.tensor.matmul(acc[:], ws[:, mi, k, :], src_t[:, k, :],
                                         start=(k == 0), stop=(k == KT - 1))
                    emit_lrelu(small, h2_t[:, m, :], acc[:], b_t["b2"][:, m:m + 1])
            src_t = h2_t

        h3_t = hpool.tile([128, MT, RPC], dt_mm, tag="h")
        for m0 in range(0, MT, 2):
            ws = wstream.tile([128, 2, KT, 128], dt_mm, tag="ws")
            nc.sync.dma_start(out=ws[:], in_=w3_d[m0:m0 + 2].rearrange("m p k c -> p m k c"))
            for mi in range(2):
                m = m0 + mi
                acc = ps.tile([128, RPC], F32, tag="acc")
                for k in range(KT):
                    nc.tensor.matmul(acc[:], ws[:, mi, k, :], h2_t[:, k, :],
                                     start=(k == 0), stop=(k == KT - 1))
                emit_lrelu(small, h3_t[:, m, :], acc[:], b_t["b3"][:, m:m + 1])

        # ---- layer 4 (swapped operands: rows on partitions) + power iteration ----
        # L4-only constants load late so their DMAs don't delay the L1/L2 weights
        w4_t = const.tile([128, KT, P], dt_mm, tag="w4")
        nc.sync.dma_start(out=w4_t[:], in_=w4_d[:])
        b4r_t = const.tile([128, P], F32, tag="b4r")
        nc.sync.dma_start(out=b4r_t[:], in_=b4r_d[:])
        msk_t = const.tile([128, 2, NN], F32, tag="msk")
        nc.sync.dma_start(out=msk_t[:], in_=msk_d.rearrange("(g p) n -> p g n", p=128))
        trow_t = const.tile([128, 2], F32, tag="trow")
        nc.sync.dma_start(out=trow_t[:], in_=trow_d.rearrange("(g p) -> p g", p=128))
        lrs = []
        for g in range(RPC // 128):
            accA = ps.tile([128, P], F32, tag="acc")
            for k in range(KT):
                nc.tensor.matmul(accA[:], h3_t[:, k, g * 128:(g + 1) * 128],
                                 w4_t[:, k, :], start=(k == 0), stop=(k == KT - 1))
            pre = small.tile([128, P], F32, tag="pre")
            nc.vector.tensor_add(pre[:], accA[:], b4r_t[:])
            lr = small.tile([128, P], F32, tag=f"lr{g}")
            emit_lrelu(small, lr[:], pre[:], 0.0)
            lrs.append(lr)
        A_ts = []
        for g in range(RPC // 128):
            A_t = small.tile([128, P], F32, tag=f"A{g}")
            nc.scalar.activation(A_t[:], lrs[g][:], mybir.ActivationFunctionType.Sigmoid)
            A_ts.append(A_t)
        for g in range(RPC // 128):
            A3 = A_ts[g][:].rearrange("p (i j) -> p i j", j=NN)

            # power iteration: v0 = ones; per-step max-normalization is folded
            # into the next multiply (the final vec/v_src ratio is
            # scale-invariant, so any per-step scaling is valid).
            v_t = pipool.tile([128, NN], F32, tag="v")
            nc.vector.memset(v_t[:], 1.0)
            ones_t = pipool.tile([128, 1], F32, tag="ones")
            nc.vector.memset(ones_t[:], 1.0)
            inv_t = ones_t
            for it in range(pi_iters):
                v_b = v_t[:].rearrange("p (a j) -> p a j", a=1).to_broadcast((128, NN, NN))
                prod = pipool.tile([128, NN, NN], F32, tag="prod")
                nc.vector.scalar_tensor_tensor(
                    prod[:], A3, inv_t[:, 0:1], v_b,
                    op0=mybir.AluOpType.mult, op1=mybir.AluOpType.mult)
                w_t = pipool.tile([128, NN], F32, tag="v")
                nc.vector.tensor_reduce(w_t[:], prod[:], axis=mybir.AxisListType.X,
                                        op=mybir.AluOpType.add)
                if it < pi_iters - 1 and it % 2 == 1:
                    mx = pipool.tile([128, 1], F32, tag="mx")
                    nc.vector.tensor_reduce(mx[:], w_t[:], axis=mybir.AxisListType.X,
                                            op=mybir.AluOpType.max)
                    inv_t = pipool.tile([128, 1], F32, tag="inv")
                    nc.vector.reciprocal(inv_t[:], mx[:])
                else:
                    inv_t = ones_t
                v_t = w_t

            # contrib = vec / vec[src] * T
            dummy = pipool.tile([128, NN], F32, tag="dummy")
            nc.vector.tensor_mul(dummy[:], v_t[:], msk_t[:, g, :])
            vsrc = pipool.tile([128, 1], F32, tag="vsrc")
            nc.vector.tensor_reduce(vsrc[:], dummy[:], axis=mybir.AxisListType.X,
                                    op=mybir.AluOpType.add)
            rsrc = pipool.tile([128, 1], F32, tag="rsrc")
            nc.vector.reciprocal(rsrc[:], vsrc[:])
            rt = pipool.tile([128, 1], F32, tag="rt")
            nc.vector.tensor_mul(rt[:], rsrc[:], trow_t[:, g:g + 1])
            contrib = pipool.tile([128, NN], F32, tag="contrib")
            nc.vector.tensor_scalar_mul(contrib[:], v_t[:], rt[:, 0:1])
            nc.sync.dma_start(out=out_d[g * 128:(g + 1) * 128, :], in_=contrib[:])

    nc.compile()
    return nc


def _np_dtype(dt_mm):
    import ml_dtypes
    if dt_mm == mybir.dt.bfloat16:
        return np.dtype(ml_dtypes.bfloat16)
    if dt_mm == mybir.dt.float16:
        return np.dtype(np.float16)
    return np.float32


def _host_prep(inputs, dt_mm):
    """Build per-core input maps. Pure layout/indexing work."""
    ndt = _np_dtype(dt_mm)
    emb = np.asarray(inputs["nodes_embeddings"], dtype=np.float32)   # (B, N, D)
    Ts = np.asarray(inputs["Ts"], dtype=np.float32)                  # (B, N, N)
    W1 = np.ascontiguousarray(
        np.asarray(inputs["W1"], np.float32).reshape(2, 128, 4, 1024).transpose(2, 1, 0, 3)
    ).astype(ndt)
    MT = KT = H // 128

    def stripe_layout(w):  # [H, H] -> [m, p, k, c] contiguous
        return np.ascontiguousarray(
            np.asarray(w, np.float32).reshape(KT, 128, MT, 128).transpose(2, 1, 0, 3)
        ).astype(ndt)

    W2 = stripe_layout(inputs["W2"])
    W3 = stripe_layout(inputs["W3"])
    W4 = np.ascontiguousarray(
        np.asarray(inputs["W4"], np.float32).reshape(KT, 128, P).transpose(1, 0, 2)
    ).astype(ndt)
    b1 = np.ascontiguousarray(np.asarray(inputs["b1"], np.float32))
    b2 = np.ascontiguousarray(np.asarray(inputs["b2"], np.float32))
    b3 = np.ascontiguousarray(np.asarray(inputs["b3"], np.float32))
    b4 = np.asarray(inputs["b4"], np.float32)
    b4r = np.ascontiguousarray(np.broadcast_to(b4[None, :], (128, P)))

    embT = emb.transpose(1, 0, 2)                       # (N, B, D)
    pair = np.arange(P)
    s_ids = pair // NN
    t_ids = pair % NN
    # faithful to the reference: stack then flat-reinterpret, NOT per-row concat
    x = np.stack([embT[s_ids], embT[t_ids]], axis=1).reshape(P, B, 2 * D)
    xr = x.reshape(P * B, 2 * D)                        # (2048, 256)

    T_pair = Ts.reshape(B, P).T                         # (P, B)
    msk_full = (np.arange(NN)[None, :] == s_ids[:, None]).astype(np.float32)  # (P, NN)

    in_maps = []
    ppc = P // NCORES                                   # 32 pairs per core
    for c in range(NCORES):
        rows = slice(c * RPC, (c + 1) * RPC)
        prs = slice(c * ppc, (c + 1) * ppc)
        xt_c = np.ascontiguousarray(xr[rows].T).astype(ndt)          # (256, RPC)
        trow_c = np.ascontiguousarray(T_pair[prs, :].reshape(RPC))   # (RPC,)
        msk_c = np.ascontiguousarray(np.repeat(msk_full[prs], B, axis=0))  # (RPC, NN)
        in_maps.append({
            "xt": xt_c, "w1": W1, "w2": W2, "w3": W3, "w4": W4,
            "b1": b1, "b2": b2, "b3": b3, "b4r": b4r,
            "trow": trow_c, "msk": msk_c,
        })
    return in_maps


def _get_program(dt_mm, sim_safe=False, l2_reps=1):
    key = (str(dt_mm), sim_safe, l2_reps)
    if key not in _cache:
        _cache[key] = _build_program(dt_mm, sim_safe=sim_safe, l2_reps=l2_reps)
    return _cache[key]


def run(inputs, dt_mm=mybir.dt.float32r, trace=False):
    nc = _get_program(dt_mm)
    in_maps = _host_prep(inputs, dt_mm)
    res = run_bass_kernel_spmd(nc, in_maps, list(range(NCORES)), trace=trace)
    contribs = [res.results[c]["contrib"].astype(np.float32) for c in range(NCORES)]
    # unshard: sum pair contributions within each core block, then across cores
    out = np.zeros((B, NN), dtype=np.float64)
    for c in range(NCORES):
        out += contribs[c].astype(np.float64).reshape(P // NCORES, B, NN).sum(axis=0)
    return out.astype(np.float32), res


def kernel(**inputs):
    out, _ = run(inputs)
    return out


# revision 15
# speedup vs baseline: 24.3518x; 1.0072x over previous
"""Trainium2 Bass kernel for the pairwise-MLP + power-iteration module.

Computation (see host reference):
  - For each of P=256 (s,t) node pairs and B=8 graphs, build a 256-d feature row
    (a flat reinterpretation of stacked s/t embeddings), run a 256->4096->4096
    ->4096->256 LeakyReLU MLP with a final sigmoid -> a 16x16 positive matrix.
  - Power-iterate each matrix to its Perron eigenvector, divide by the source
    component, scale by Ts[pair], and sum contributions over all pairs -> (B, N).

Distribution: data-parallel over the P axis; each of the 8 cores runs 32 pairs
x 8 graphs = 256 MLP rows. Activations stay SBUF-resident in transposed layout
(hidden on partitions, rows on the free axis); W2/W3 stream from HBM in
column-stripes. Matmuls run in float32r (TF32-like multiply, fp32 accumulate).
The per-core [256 rows, 16] contribution block is returned and the final
pair/core reduction happens on the host (the all-reduce step).
"""

import numpy as np
from contextlib import ExitStack

import concourse.bass as bass
import concourse.tile as tile
from concourse import bacc, mybir
from concourse.bass_utils import run_bass_kernel_spmd

B = 8
NN = 16
D = 128
H = 4096
P = NN * NN            # 256 pairs
NCORES = 8
RPC = P * B // NCORES  # 256 rows per core
SLOPE = 0.01
PI_ITERS = 8

F32 = mybir.dt.float32

_cache = {}


def _build_program(dt_mm, pi_iters=PI_ITERS, sim_safe=False, l2_reps=1):
    KT1 = 2 * D // 128   # 2 k-tiles for layer 1
    KT = H // 128        # 32 k-tiles for layers 2-4
    MT = H // 128        # 32 m-stripes for layers 1-3
    MT_G, KT_G = MT, KT
    KT1_G = 2 * D // 128

    nc = bacc.Bacc("TRN2", target_bir_lowering=False, debug=False,
                   num_devices=NCORES)

    def emit_lrelu(pool, out_ap, in_ap, bias):
        """out = LeakyReLU(in + bias). CoreSim lacks Lrelu, so the sim build
        decomposes it as pre*alpha + relu(pre)*(1-alpha)."""
        if not sim_safe:
            nc.scalar.activation(out_ap, in_ap, mybir.ActivationFunctionType.Lrelu,
                                 bias=bias, scale=1.0, alpha=SLOPE)
            return
        shape = [in_ap.shape[0], in_ap.free_size()]
        pre = pool.tile(shape, F32, tag="lr_pre")
        nc.scalar.activation(pre[:], in_ap, mybir.ActivationFunctionType.Identity,
                             bias=bias, scale=1.0)
        pos = pool.tile(shape, F32, tag="lr_pos")
        nc.scalar.activation(pos[:], pre[:], mybir.ActivationFunctionType.Relu)
        t1 = pool.tile(shape, F32, tag="lr_t1")
        nc.vector.tensor_scalar_mul(t1[:], pos[:], 1.0 - SLOPE)
        nc.vector.scalar_tensor_tensor(out_ap, pre[:], SLOPE, t1[:],
                                       op0=mybir.AluOpType.mult,
                                       op1=mybir.AluOpType.add)

    xt_d = nc.dram_tensor("xt", [2 * D, RPC], dt_mm, kind="ExternalInput").ap()
    w1_d = nc.dram_tensor("w1", [4, 128, KT1_G, 8 * 128], dt_mm, kind="ExternalInput").ap()
    # stripe-major host-rearranged layouts: one contiguous block per m-stripe,
    # 16KB+ contiguous per partition line (keeps DMA descriptors large)
    w2_d = nc.dram_tensor("w2", [MT_G, 128, KT_G, 128], dt_mm, kind="ExternalInput").ap()
    w3_d = nc.dram_tensor("w3", [MT_G, 128, KT_G, 128], dt_mm, kind="ExternalInput").ap()
    w4_d = nc.dram_tensor("w4", [128, KT_G, P], dt_mm, kind="ExternalInput").ap()
    b1_d = nc.dram_tensor("b1", [H], F32, kind="ExternalInput").ap()
    b2_d = nc.dram_tensor("b2", [H], F32, kind="ExternalInput").ap()
    b3_d = nc.dram_tensor("b3", [H], F32, kind="ExternalInput").ap()
    b4r_d = nc.dram_tensor("b4r", [128, P], F32, kind="ExternalInput").ap()
    trow_d = nc.dram_tensor("trow", [RPC], F32, kind="ExternalInput").ap()
    msk_d = nc.dram_tensor("msk", [RPC, NN], F32, kind="ExternalInput").ap()
    out_d = nc.dram_tensor("contrib", [RPC, NN], F32, kind="ExternalOutput").ap()

    with tile.TileContext(nc) as tc, ExitStack() as ctx:
        esz = 2 if dt_mm in (mybir.dt.float16, mybir.dt.bfloat16) else 4
        ws_bufs = 4 if esz == 2 else 2
        const = ctx.enter_context(tc.tile_pool(name="const", bufs=1))
        hpool = ctx.enter_context(tc.tile_pool(name="h", bufs=2))
        wstream = ctx.enter_context(tc.tile_pool(name="wstream", bufs=ws_bufs))
        small = ctx.enter_context(tc.tile_pool(name="small", bufs=2))
        pipool = ctx.enter_context(tc.tile_pool(name="pi", bufs=2))
        ps = ctx.enter_context(tc.tile_pool(name="ps", bufs=8, space="PSUM"))

        # ---- resident loads ----
        xt_t = const.tile([128, KT1, RPC], dt_mm, tag="xt")
        nc.sync.dma_start(out=xt_t[:], in_=xt_d.rearrange("(k p) r -> p k r", p=128))
        b_t = {}
        for nm, bd in [("b1", b1_d), ("b2", b2_d), ("b3", b3_d)]:
            t = const.tile([128, MT], F32, tag=nm)
            nc.sync.dma_start(out=t[:], in_=bd.rearrange("(m p) -> p m", p=128))
            b_t[nm] = t

        # ---- MLP layers 1-3 (output transposed: hidden on partitions) ----


        h1_t = hpool.tile([128, MT, RPC], dt_mm, tag="h")
        w1_chunks = []
        for ch in range(4):
            w1c = const.tile([128, KT1, 8 * 128], dt_mm, tag=f"w1c{ch}")
            nc.sync.dma_start(out=w1c[:], in_=w1_d[ch])
            w1_chunks.append(w1c)
        for m in range(MT):
            w1c = w1_chunks[m // 8]
            mi = m % 8
            acc = ps.tile([128, RPC], F32, tag="acc")
            for k in range(KT1):
                nc.tensor.matmul(acc[:], w1c[:, k, mi * 128:(mi + 1) * 128],
                                 xt_t[:, k, :], start=(k == 0), stop=(k == KT1 - 1))
            emit_lrelu(small, h1_t[:, m, :], acc[:], b_t["b1"][:, m:m + 1])

        src_t = h1_t
        for rep in range(l2_reps):
            h2_t = hpool.tile([128, MT, RPC], dt_mm, tag="h")
            for m0 in range(0, MT, 2):
                ws = wstream.tile([128, 2, KT, 128], dt_mm, tag="ws")
                nc.sync.dma_start(out=ws[:], in_=w2_d[m0:m0 + 2].rearrange("m p k c -> p m k c"))
                for mi in range(2):
                    m = m0 + mi
                    acc = ps.tile([128, RPC], F32, tag="acc")
                    for k in range(KT):
                        nc.tensor.matmul(acc[:], ws[:, mi, k, :], src_t[:, k, :],
                                         start=(k == 0), stop=(k == KT - 1))
                    emit_lrelu(small, h2_t[:, m, :], acc[:], b_t["b2"][:, m:m + 1])
            src_t = h2_t

        h3_t = hpool.tile([128, MT, RPC], dt_mm, tag="h")
        for m0 in range(0, MT, 2):
            ws = wstream.tile([128, 2, KT, 128], dt_mm, tag="ws")
            nc.sync.dma_start(out=ws[:], in_=w3_d[m0:m0 + 2].rearrange("m p k c -> p m k c"))
            for mi in range(2):
                m = m0 + mi
                acc = ps.tile([128, RPC], F32, tag="acc")
                for k in range(KT):
                    nc.tensor.matmul(acc[:], ws[:, mi, k, :], h2_t[:, k, :],
                                     start=(k == 0), stop=(k == KT - 1))
                emit_lrelu(small, h3_t[:, m, :], acc[:], b_t["b3"][:, m:m + 1])

        # ---- layer 4 (swapped operands: rows on partitions) + power iteration ----
        # L4-only constants load late so their DMAs don't delay the L1/L2 weights
        w4_t = const.tile([128, KT, P], dt_mm, tag="w4")
        nc.sync.dma_start(out=w4_t[:], in_=w4_d[:])
        b4r_t = const.tile([128, P], F32, tag="b4r")
        nc.sync.dma_start(out=b4r_t[:], in_=b4r_d[:])
        msk_t = const.tile([128, 2, NN], F32, tag="msk")
        nc.sync.dma_start(out=msk_t[:], in_=msk_d.rearrange("(g p) n -> p g n", p=128))
        trow_t = const.tile([128, 2], F32, tag="trow")
        nc.sync.dma_start(out=trow_t[:], in_=trow_d.rearrange("(g p) -> p g", p=128))
        lrs = []
        for g in range(RPC // 128):
            accA = ps.tile([128, P], F32, tag="acc")
            for k in range(KT):
                nc.tensor.matmul(accA[:], h3_t[:, k, g * 128:(g + 1) * 128],
                                 w4_t[:, k, :], start=(k == 0), stop=(k == KT - 1))
            pre = small.tile([128, P], F32, tag="pre")
            nc.vector.tensor_add(pre[:], accA[:], b4r_t[:])
            lr = small.tile([128, P], F32, tag=f"lr{g}")
            emit_lrelu(small, lr[:], pre[:], 0.0)
            lrs.append(lr)
        A_ts = []
        for g in range(RPC // 128):
            A_t = small.tile([128, P], F32, tag=f"A{g}")
            nc.scalar.activation(A_t[:], lrs[g][:], mybir.ActivationFunctionType.Sigmoid)
            A_ts.append(A_t)
        for g in range(RPC // 128):
            A3 = A_ts[g][:].rearrange("p (i j) -> p i j", j=NN)

            # power iteration: v0 = ones; per-step max-normalization is folded
            # into the next multiply (the final vec/v_src ratio is
            # scale-invariant, so any per-step scaling is valid).
            v_t = pipool.tile([128, NN], F32, tag="v")
            nc.vector.memset(v_t[:], 1.0)
            ones_t = pipool.tile([128, 1], F32, tag="ones")
            nc.vector.memset(ones_t[:], 1.0)
            inv_t = ones_t
            for it in range(pi_iters):
                v_b = v_t[:].rearrange("p (a j) -> p a j", a=1).to_broadcast((128, NN, NN))
                prod = pipool.tile([128, NN, NN], F32, tag="prod")
                nc.vector.scalar_tensor_tensor(
                    prod[:], A3, inv_t[:, 0:1], v_b,
                    op0=mybir.AluOpType.mult, op1=mybir.AluOpType.mult)
                w_t = pipool.tile([128, NN], F32, tag="v")
                nc.vector.tensor_reduce(w_t[:], prod[:], axis=mybir.AxisListType.X,
                                        op=mybir.AluOpType.add)
                if it < pi_iters - 1 and it % 2 == 1:
                    mx = pipool.tile([128, 1], F32, tag="mx")
                    nc.vector.tensor_reduce(mx[:], w_t[:], axis=mybir.AxisListType.X,
                                            op=mybir.AluOpType.max)
                    inv_t = pipool.tile([128, 1], F32, tag="inv")
                    nc.vector.reciprocal(inv_t[:], mx[:])
                else:
                    inv_t = ones_t
                v_t = w_t

            # contrib = vec / vec[src] * T
            dummy = pipool.tile([128, NN], F32, tag="dummy")
            nc.vector.tensor_mul(dummy[:], v_t[:], msk_t[:, g, :])
            vsrc = pipool.tile([128, 1], F32, tag="vsrc")
            nc.vector.tensor_reduce(vsrc[:], dummy[:], axis=mybir.AxisListType.X,
                                    op=mybir.AluOpType.add)
            rsrc = pipool.tile([128, 1], F32, tag="rsrc")
            nc.vector.reciprocal(rsrc[:], vsrc[:])
            rt = pipool.tile([128, 1], F32, tag="rt")
            nc.vector.tensor_mul(rt[:], rsrc[:], trow_t[:, g:g + 1])
            contrib = pipool.tile([128, NN], F32, tag="contrib")
            nc.vector.tensor_scalar_mul(contrib[:], v_t[:], rt[:, 0:1])
            nc.sync.dma_start(out=out_d[g * 128:(g + 1) * 128, :], in_=contrib[:])

    nc.compile()
    return nc


def _np_dtype(dt_mm):
    import ml_dtypes
    if dt_mm == mybir.dt.bfloat16:
        return np.dtype(ml_dtypes.bfloat16)
    if dt_mm == mybir.dt.float16:
        return np.dtype(np.float16)
    return np.float32


def _host_prep(inputs, dt_mm):
    """Build per-core input maps. Pure layout/indexing work."""
    ndt = _np_dtype(dt_mm)
    emb = np.asarray(inputs["nodes_embeddings"], dtype=np.float32)   # (B, N, D)
    Ts = np.asarray(inputs["Ts"], dtype=np.float32)                  # (B, N, N)
    W1 = np.ascontiguousarray(
        np.asarray(inputs["W1"], np.float32).reshape(2, 128, 4, 1024).transpose(2, 1, 0, 3)
    ).astype(ndt)
    MT = KT = H // 128

    def stripe_layout(w):  # [H, H] -> [m, p, k, c] contiguous
        return np.ascontiguousarray(
            np.asarray(w, np.float32).reshape(KT, 128, MT, 128).transpose(2, 1, 0, 3)
        ).astype(ndt)

    W2 = stripe_layout(inputs["W2"])
    W3 = stripe_layout(inputs["W3"])
    W4 = np.ascontiguousarray(
        np.asarray(inputs["W4"], np.float32).reshape(KT, 128, P).transpose(1, 0, 2)
    ).astype(ndt)
    b1 = np.ascontiguousarray(np.asarray(inputs["b1"], np.float32))
    b2 = np.ascontiguousarray(np.asarray(inputs["b2"], np.float32))
    b3 = np.ascontiguousarray(np.asarray(inputs["b3"], np.float32))
    b4 = np.asarray(inputs["b4"], np.float32)
    b4r = np.ascontiguousarray(np.broadcast_to(b4[None, :], (128, P)))

    embT = emb.transpose(1, 0, 2)                       # (N, B, D)
    pair = np.arange(P)
    s_ids = pair // NN
    t_ids = pair % NN
    # faithful to the reference: stack then flat-reinterpret, NOT per-row concat
    x = np.stack([embT[s_ids], embT[t_ids]], axis=1).reshape(P, B, 2 * D)
    xr = x.reshape(P * B, 2 * D)                        # (2048, 256)

    T_pair = Ts.reshape(B, P).T                         # (P, B)
    msk_full = (np.arange(NN)[None, :] == s_ids[:, None]).astype(np.float32)  # (P, NN)

    in_maps = []
    ppc = P // NCORES                                   # 32 pairs per core
    for c in range(NCORES):
        rows = slice(c * RPC, (c + 1) * RPC)
        prs = slice(c * ppc, (c + 1) * ppc)
        xt_c = np.ascontiguousarray(xr[rows].T).astype(ndt)          # (256, RPC)
        trow_c = np.ascontiguousarray(T_pair[prs, :].reshape(RPC))   # (RPC,)
        msk_c = np.ascontiguousarray(np.repeat(msk_full[prs], B, axis=0))  # (RPC, NN)
        in_maps.append({
            "xt": xt_c, "w1": W1, "w2": W2, "w3": W3, "w4": W4,
            "b1": b1, "b2": b2, "b3": b3, "b4r": b4r,
            "trow": trow_c, "msk": msk_c,
        })
    return in_maps


def _get_program(dt_mm, sim_safe=False, l2_reps=1):
    key = (str(dt_mm), sim_safe, l2_reps)
    if key not in _cache:
        _cache[key] = _build_program(dt_mm, sim_safe=sim_safe, l2_reps=l2_reps)
    return _cache[key]


def run(inputs, dt_mm=mybir.dt.float32r, trace=False):
    nc = _get_program(dt_mm)
    in_maps = _host_prep(inputs, dt_mm)
    res = run_bass_kernel_spmd(nc, in_maps, list(range(NCORES)), trace=trace)
    contribs = [res.results[c]["contrib"].astype(np.float32) for c in range(NCORES)]
    # unshard: sum pair contributions within each core block, then across cores
    out = np.zeros((B, NN), dtype=np.float64)
    for c in range(NCORES):
        out += contribs[c].astype(np.float64).reshape(P // NCORES, B, NN).sum(axis=0)
    return out.astype(np.float32), res


def kernel(**inputs):
    out, _ = run(inputs)
    return out
